# revision 42
# baseline (speedup 1.0000x reference)
"""APPNP_Net Trainium2 kernel (8 NeuronCores, SPMD row-sharded), fp8 edition.

The reference model is:
    h = relu(x @ W1 + b1); z = h @ W2 + b2; out = log_softmax(z, axis=1)
followed by K=10 APPNP propagation steps with ALPHA=1.0.  Since
z_{t+1} = (1-ALPHA)*agg + ALPHA*h == h, the propagation is the identity
and edge_index never affects the output.  So the kernel is a row-wise
MLP + log_softmax, sharded by nodes across the 8 cores.

v4 changes (52.7us -> target ~46us), from NTFF trace analysis of v3:
  - x is packed block-major on the host ([pair, sub, p, k, r]), so every
    per-block DMA slice moves 1-2KB contiguous partition lines instead
    of the 512B strided runs that made block0's k01 land at 11.2us.
  - head descriptors split across both HWDGE queues: scalar writes
    block0-k01's descriptor (its first op, before the ACT_TABLE_LOAD),
    sync writes W1 / block0-k23 / biases / block1 / pair1.  ~2us earlier
    first matmul.
  - 10 warm-up matmuls (16 overshot: the last ones delayed the real
    stream; HAM's SHORT window kept resetting across the data-wait gap).
  - 12 x bufs (6 pairs in flight): v3's 1.9+0.9us PE gaps at 41-44us
    were the DMA rate-matching the shallow ring, then falling behind.
  - z-chain fused per PAIR: both blocks' MM2 outputs land in ONE psum
    bank ([128,2,4,50] = 1.6KB), so scale+bias / exp / row-sum /
    broadcast-subtract each run once per pair on 2x the elements —
    per-op overhead (~0.1-0.3us each on ACT/DVE/GpSimd) halves.
    ACT busy 31.4 -> ~26, DVE 31.5 -> ~24 predicted.

Numerics (unchanged): fp8e4 DoubleRow MM1 with W1'=16*W1, b1'=16*b1;
MM2 is plain fp8 (FD=50 < 128 makes DoubleRow's interleaved LDWEIGHTS a
net loss; FWL loads are ~50ns) with W2'=4*W2 so pz=64*(z-b2); t = pz/64
+ b2 fused in one DVE scalar_tensor_tensor; ACT table pinned; x fp8,
outputs bf16.  Rel err ~3.7e-3 vs the 2e-2 gate.
"""

import sys

sys.path.insert(0, "/opt/trn_rl_repo")

import dataclasses
from contextlib import ExitStack

import numpy as np
import ml_dtypes

import bass_rust as _bass_rust

import concourse.tile as tile
from concourse import bacc, mybir
from concourse.bass_utils import run_bass_kernel_spmd
from concourse.hw_specs import get_activation_tables

# NOTE on two dead ends, for future sessions: (1) walrus
# --enable-ldw-opt=true (redundant-LDWEIGHTS dedupe) rejects bass kernels
# (bass always emits standalone InstLdweights).  (2) capping walrus
# --max-sem-num to shrink the ~8.1us end-of-kernel semaphore-clear epilogue
# REGRESSES ~9us: fewer semaphores serialize the DMA queues, and the
# epilogue (a fixed 253-clear sweep split across the 5 engines) doesn't
# shrink.  The epilogue is ucode-fixed overhead inside the measured window.

N_NODES = 100000
F_IN = 512
HID = 256
C = 50
N_CORES = 8
BLOCK = 512
NBLK = 25  # 24 full blocks + 1 half block
NPAIR = 12  # full-block pairs
ROWS_PER_CORE = 12544  # 98 * 128; 8 * 12544 = 100352 >= 100000 (zero-padded)
TAIL_ROWS = ROWS_PER_CORE - NPAIR * 2 * BLOCK  # 256
# log-softmax groups (in blocks; boundaries must be even so pairs don't
# straddle groups): late groups are small so their serial ln/subtract
# chains overlap the remaining PE work instead of trailing it
GROUPS = [(0, 8), (8, 16), (16, 20), (20, 24), (24, 25)]
KC = F_IN // 128  # 4 contraction chunks for MM1
MH = HID // 128  # 2 hidden chunks
SUB = BLOCK // 128  # 4 row-subtiles per full block
NSUB = ROWS_PER_CORE // 128  # 98
N_WARM = 10  # dummy PE matmuls riding out the head DMA (~1.3us)

W1_SCALE = 16.0  # W1, b1 pre-scaled by this on host (fp8 normal range)
W2_SCALE = 4.0  # W2 pre-scaled by this on host
PZ_INV = 1.0 / (W1_SCALE * W2_SCALE)  # pz = 64*(z-b2); t = pz*PZ_INV + b2

BF16 = mybir.dt.bfloat16
F32 = mybir.dt.float32
FP8 = mybir.dt.float8e4
DR = mybir.MatmulPerfMode.DoubleRow
np_bf16 = ml_dtypes.bfloat16
np_fp8 = ml_dtypes.float8_e4m3

_GROUP_OF = {}
for _gi, (_s, _e) in enumerate(GROUPS):
    for _b in range(_s, _e):
        _GROUP_OF[_b] = _gi


def _bcast_cols(ap2d, reps):
    """[P, Q] AP -> [P, Q, reps] AP with a zero-stride inner dim."""
    return dataclasses.replace(ap2d, ap=[ap2d.ap[0], ap2d.ap[1], [0, reps]])


def _rep_dim(ap3d, reps):
    """[P, A, B] AP -> [P, reps, A, B] AP with a zero-stride leading dim."""
    return dataclasses.replace(
        ap3d, ap=[ap3d.ap[0], [0, reps], ap3d.ap[1], ap3d.ap[2]]
    )


def _pin_act_table(nc):
    """Constrain the ACT-table placement pass to natural_log_exp_and_others,
    which serves Relu, Exp AND Ln at full (400-bucket) resolution.  The
    default fixpoint picks exp_and_others for the steady Relu/Exp loop and
    switches tables around every Ln, costing ~1.3us per ACT_TABLE_LOAD; one
    table means one load.  Indices into the full list are preserved (the
    emitted act_func_set_id indexes act_info.json's act_func_sets)."""

    def patched():
        tables = [
            (name, funcs if name == "natural_log_exp_and_others" else set())
            for name, funcs in get_activation_tables(nc.m.arch).items()
        ]
        _bass_rust.insert_act_table_loads(nc, tables)

    nc.insert_act_table_loads = patched


def build_nc():
    nc = bacc.Bacc(
        "TRN2",
        target_bir_lowering=False,
        debug=False,
        num_devices=N_CORES,
    )
    _pin_act_table(nc)
    # [pair, p, sub, k, r]: a whole-pair transfer is 4KB contiguous per
    # partition line, per-block head slices are 1-2KB contiguous, and no
    # permuted DMA access patterns are needed anywhere
    xT = nc.declare_dram_parameter(
        "xT", [NPAIR, 128, 2, KC, BLOCK], FP8, isOutput=False
    ).ap()
    xTt = nc.declare_dram_parameter(
        "xTt", [128, KC, TAIL_ROWS], FP8, isOutput=False
    ).ap()
    # W1 packed [p, k, hid] (x16); W2 packed [p, kh, C] (x4); biases
    # packed [p, MH + SUB*C] (16*b1 columns then b2 broadcast)
    W1p = nc.declare_dram_parameter("W1p", [128, KC, HID], FP8, isOutput=False).ap()
    W2p = nc.declare_dram_parameter("W2p", [128, MH, C], FP8, isOutput=False).ap()
    bc = nc.declare_dram_parameter("bc", [128, MH + SUB * C], F32, isOutput=False).ap()
    # out[p, q, c] with row = q*128 + p (host transposes back)
    out = nc.declare_dram_parameter("out", [128, NSUB, C], BF16, isOutput=True).ap()

    with tile.TileContext(nc) as tc, ExitStack() as ctx:
        consts = ctx.enter_context(tc.tile_pool(name="consts", bufs=1))
        # 13 x bufs: pairs 0-11 + the tail each get a slot, so every x
        # descriptor can be written before the first out-store descriptor.
        # Store descriptors carry semaphore WAITS and block the sync queue
        # until their subtract fires — pair 11's x descriptor emitted after
        # them executed only at ~31us, starving the PE at ~36us for 1.7us.
        xpool = ctx.enter_context(tc.tile_pool(name="x", bufs=13))
        hpool = ctx.enter_context(tc.tile_pool(name="h", bufs=4))
        tpool = ctx.enter_context(tc.tile_pool(name="t", bufs=3))
        epool = ctx.enter_context(tc.tile_pool(name="e", bufs=3))
        spool = ctx.enter_context(tc.tile_pool(name="s", bufs=2))
        opool = ctx.enter_context(tc.tile_pool(name="o", bufs=4))
        # ph0/ph1 rings hold MM1 accumulators (6 banks); the pz ring holds
        # MM2 pair-outputs (1 bank each: 2*4*50 fp32 = 1.6KB) + warm-up
        psum = ctx.enter_context(tc.tile_pool(name="psum", bufs=3, space="PSUM"))
        psumz = ctx.enter_context(tc.tile_pool(name="psumz", bufs=2, space="PSUM"))

        # ── PE warm-up ────────────────────────────────────────────────
        # HAM throttles an idle PE to 1.2 GHz and needs ~3.4us of sustained
        # matmul activity to lift.  Burn the head's DMA wait with dummy DR
        # matmuls on a memset scratch so the real MM1 stream starts warm-ish.
        warm = consts.tile([128, 2, 128], FP8, tag="warm")
        nc.vector.memset(warm, 0.25)
        wps = psumz.tile([128, 128], F32, tag="pz", name="warm_ps")
        for _ in range(N_WARM):
            nc.tensor.matmul(
                wps, lhsT=warm, rhs=warm, start=True, stop=True, perf_mode=DR
            )

        # ── head DMAs, split across both HWDGE queues ─────────────────
        # DIRECT2D descriptor writes cost ~0.65us each and serialize per
        # engine.  scalar's first op is block0-k01's descriptor (the first
        # matmul's gating data, in flight by ~8us); sync carries W1 then
        # the rest in need-order.  gpsimd's SWDGE queue stays unused.
        # sync carries the x stream in need-order (k01, k23, block1,
        # pair1, ...) so every chunk's descriptor lands ~0.65us earlier
        # than with W1 in front; scalar carries the small consts (W1
        # 128KB, biases, W2) whose transfers finish almost instantly.
        xts = {}
        xt0 = xpool.tile([128, 2, KC, BLOCK], FP8, tag="xt", name="xt0")
        nc.sync.dma_start(out=xt0[:, 0, :2, :], in_=xT[0, :, 0, :2, :])
        w1t = consts.tile([128, KC, HID], FP8, tag="w1")
        nc.scalar.dma_start(out=w1t, in_=W1p)
        nc.sync.dma_start(out=xt0[:, 0, 2:, :], in_=xT[0, :, 0, 2:, :])
        bct = consts.tile([128, MH + SUB * C], F32, tag="bc")
        nc.scalar.dma_start(out=bct, in_=bc)
        nc.sync.dma_start(out=xt0[:, 1], in_=xT[0, :, 1])
        b1sb = bct[:, :MH]
        b2sb = bct[:, MH:].rearrange("p (s c) -> p s c", s=SUB)
        w2t = consts.tile([128, MH, C], FP8, tag="w2")
        nc.scalar.dma_start(out=w2t, in_=W2p)
        xts[0] = xt0
        xt1 = xpool.tile([128, 2, KC, BLOCK], FP8, tag="xt", name="xt1")
        nc.sync.dma_start(out=xt1, in_=xT[1])
        xts[1] = xt1

        def issue_pair(p):
            if p == NPAIR:
                xt = xpool.tile([128, KC, TAIL_ROWS], FP8, tag="xt", name="xt_tail")
                nc.sync.dma_start(out=xt, in_=xTt)
            else:
                xt = xpool.tile([128, 2, KC, BLOCK], FP8, tag="xt", name=f"xt{p}")
                nc.sync.dma_start(out=xt, in_=xT[p])
            xts[p] = xt

        hs = {}  # block -> h tile [128, MH, BLOCK] fp8 (16*relu(...))
        t_gs = {}  # group -> t tile
        s_gs = {}  # group -> s tile

        # With the ACT table pinned (no Ln switches), relu routing is pure
        # load balance: ACT takes mh0 always, plus mh1 on a few blocks so
        # ACT (~relu0+exp+ln) and DVE (~relu1+scale-bias+reduce) even out.
        # block 23's relu on ACT relieves DVE right where MM2(p11)/MM2(tail)
        # were stalling ~1.3us on relu semaphores at the end of the stream
        act_mh1 = {5, 9, 13, 23}

        def emit_relu(b, mh, ph, R):
            ht = hs[b]
            if mh == 0 or b in act_mh1:
                # ScalarE: 16h = relu(ph + 16*b1)
                nc.scalar.activation(
                    ht[:, mh, :R],
                    ph[:, :R],
                    mybir.ActivationFunctionType.Relu,
                    bias=b1sb[:, mh : mh + 1],
                )
            else:
                # VectorE: (ph + 16*b1) max 0 — balance the engines
                nc.vector.tensor_scalar(
                    out=ht[:, mh, :R],
                    in0=ph[:, :R],
                    scalar1=b1sb[:, mh : mh + 1],
                    scalar2=0.0,
                    op0=mybir.AluOpType.add,
                    op1=mybir.AluOpType.max,
                )

        def emit_pair_front(p):
            """x prefetch, MM1 (fp8 DR, mh->k2->block), relu for pair p."""
            for pf in range(2, min(p + 8, NPAIR + 1)):
                if pf not in xts:
                    issue_pair(pf)
            xt = xts[p]
            blocks = [2 * p, 2 * p + 1]
            for b in blocks:
                hs[b] = hpool.tile([128, MH, BLOCK], FP8, tag="h", name=f"h{b}")
            for mh in range(MH):
                phs = [
                    psum.tile([128, BLOCK], F32, tag=f"ph{mh}", name=f"ph{mh}_{b}")
                    for b in blocks
                ]
                for k2 in range(KC // 2):
                    for s in range(2):
                        nc.tensor.matmul(
                            phs[s],
                            lhsT=w1t[:, 2 * k2 : 2 * k2 + 2, mh * 128 : (mh + 1) * 128],
                            rhs=xt[:, s, 2 * k2 : 2 * k2 + 2, :],
                            start=(k2 == 0),
                            stop=(k2 == KC // 2 - 1),
                            perf_mode=DR,
                        )
                for s, b in enumerate(blocks):
                    emit_relu(b, mh, phs[s], BLOCK)
                if p < 2:
                    # bridge the early x-DMA gaps (pair1/2 land ~12-14us)
                    # with dependency-free dummies so HAM's 3.4us
                    # continuous-activity window isn't reset — each reset
                    # costs ~1.5-3us of half-clock PE time; a dummy costs
                    # ~107ns only if the real data was late anyway
                    for _ in range(3):
                        nc.tensor.matmul(
                            wps, lhsT=warm, rhs=warm,
                            start=True, stop=True, perf_mode=DR,
                        )

        def emit_tail_front():
            """MM1 + relu for the final half block (256 rows)."""
            b = NBLK - 1
            R = TAIL_ROWS
            if NPAIR not in xts:
                issue_pair(NPAIR)
            xt = xts[NPAIR]
            hs[b] = hpool.tile([128, MH, BLOCK], FP8, tag="h", name=f"h{b}")
            for mh in range(MH):
                ph = psum.tile([128, BLOCK], F32, tag=f"ph{mh}", name=f"ph{mh}_t")
                for k2 in range(KC // 2):
                    nc.tensor.matmul(
                        ph[:, :R],
                        lhsT=w1t[:, 2 * k2 : 2 * k2 + 2, mh * 128 : (mh + 1) * 128],
                        rhs=xt[:, 2 * k2 : 2 * k2 + 2, :R],
                        start=(k2 == 0),
                        stop=(k2 == KC // 2 - 1),
                        perf_mode=DR,
                    )
                emit_relu(b, mh, ph, R)

        def _group_tiles(g):
            g0, g1 = GROUPS[g]
            glen = g1 - g0
            t_gs[g] = tpool.tile([128, glen, SUB, C], BF16, tag="t", name=f"t_g{g}")
            s_gs[g] = spool.tile([128, glen * SUB], BF16, tag="s", name=f"s_g{g}")

        def emit_back_pair(p):
            """MM2 (both blocks into ONE psum bank), then pair-granular
            scale+bias, exp, row-sum.  Halves the per-op overhead vs
            per-block ops."""
            b0 = 2 * p
            g = _GROUP_OF[b0]
            g0, g1 = GROUPS[g]
            j = b0 - g0
            if j == 0:
                _group_tiles(g)
            t_g, s_g = t_gs[g], s_gs[g]
            pz = psumz.tile([128, 2, SUB, C], F32, tag="pz", name=f"pz{p}")
            for s in range(2):
                ht = hs.pop(b0 + s)
                # MM2 runs non-DR: at FD=50 DoubleRow's interleaved
                # LDWEIGHTS (~140ns) loses to FWL (~50ns); two accumulating
                # 128-contraction matmuls per subtile
                for rs in range(SUB):
                    for mh in range(MH):
                        nc.tensor.matmul(
                            pz[:, s, rs, :],
                            lhsT=ht[:, mh, rs * 128 : (rs + 1) * 128],
                            rhs=w2t[:, mh, :],
                            start=(mh == 0),
                            stop=(mh == MH - 1),
                        )
            # t = z = pz/64 + b2 for both blocks in one DVE op
            nc.vector.scalar_tensor_tensor(
                out=t_g[:, j : j + 2],
                in0=pz,
                scalar=PZ_INV,
                in1=_rep_dim(b2sb, 2),
                op0=mybir.AluOpType.mult,
                op1=mybir.AluOpType.add,
            )
            e = epool.tile([128, 2, SUB, C], BF16, tag="e", name=f"e{p}")
            nc.scalar.activation(
                e, t_g[:, j : j + 2], mybir.ActivationFunctionType.Exp
            )
            with nc.allow_low_precision(reason="bf16 softmax row-sum"):
                nc.vector.reduce_sum(
                    out=s_g[:, j * SUB : (j + 2) * SUB].rearrange(
                        "p (a s) -> p a s", a=2
                    ),
                    in_=e,
                    axis=mybir.AxisListType.X,
                )

        def emit_back_tail():
            """MM2 + z-chain for the final half block."""
            b = NBLK - 1
            S = TAIL_ROWS // 128
            g = _GROUP_OF[b]
            g0, _ = GROUPS[g]
            j = b - g0
            if j == 0:
                _group_tiles(g)
            t_g, s_g = t_gs[g], s_gs[g]
            ht = hs.pop(b)
            pz = psumz.tile([128, SUB, C], F32, tag="pz", name="pz_t")
            for rs in range(S):
                for mh in range(MH):
                    nc.tensor.matmul(
                        pz[:, rs, :],
                        lhsT=ht[:, mh, rs * 128 : (rs + 1) * 128],
                        rhs=w2t[:, mh, :],
                        start=(mh == 0),
                        stop=(mh == MH - 1),
                    )
            nc.vector.scalar_tensor_tensor(
                out=t_g[:, j, :S, :],
                in0=pz[:, :S, :],
                scalar=PZ_INV,
                in1=b2sb[:, :S, :],
                op0=mybir.AluOpType.mult,
                op1=mybir.AluOpType.add,
            )
            e = epool.tile([128, SUB, C], BF16, tag="e", name="e_t")
            nc.scalar.activation(
                e[:, :S, :], t_g[:, j, :S, :], mybir.ActivationFunctionType.Exp
            )
            with nc.allow_low_precision(reason="bf16 softmax row-sum"):
                nc.vector.reduce_sum(
                    out=s_g[:, j * SUB : j * SUB + S],
                    in_=e[:, :S, :],
                    axis=mybir.AxisListType.X,
                )

        def emit_group_tail(g):
            """ls = ln(s); out = z - ls (class-broadcast) per pair; store."""
            g0, g1 = GROUPS[g]
            t_g, s_g = t_gs.pop(g), s_gs.pop(g)
            last = g1 == NBLK
            ncols = (g1 - g0 - 1) * SUB + (TAIL_ROWS // 128 if last else SUB)
            ls_g = spool.tile(
                [128, (g1 - g0) * SUB], BF16, tag="ls", name=f"ls_g{g}"
            )
            nc.scalar.activation(
                ls_g[:, :ncols], s_g[:, :ncols], mybir.ActivationFunctionType.Ln
            )
            for j in range(0, g1 - g0, 2):
                b = g0 + j
                pair = b // 2
                if b == NBLK - 1:  # half-block tail, alone in its group
                    S = TAIL_ROWS // 128
                    zo = opool.tile([128, S, C], BF16, tag="zo", name="zo_t")
                    nc.vector.tensor_tensor(
                        out=zo,
                        in0=t_g[:, j, :S, :],
                        in1=_bcast_cols(ls_g[:, j * SUB : j * SUB + S], C),
                        op=mybir.AluOpType.subtract,
                    )
                    # sync is idle by now; scalar carries pair 11's store
                    nc.sync.dma_start(
                        out=out[:, pair * 2 * SUB : pair * 2 * SUB + S, :], in_=zo
                    )
                    continue
                zo = opool.tile([128, 2 * SUB, C], BF16, tag="zo", name=f"zo{pair}")
                if g == len(GROUPS) - 2 and j >= g1 - g0 - 2:
                    sub_engine = nc.vector  # last full pair: DVE is freeing up
                else:
                    sub_engine = nc.gpsimd  # mid-stream: keep DVE for relu
                sub_engine.tensor_tensor(
                    out=zo,
                    in0=t_g[:, j : j + 2].rearrange("p a s c -> p (a s) c"),
                    in1=_bcast_cols(ls_g[:, j * SUB : (j + 2) * SUB], C),
                    op=mybir.AluOpType.subtract,
                )
                q0 = pair * 2 * SUB
                # spread the three final store descriptors (p10, p11, tail)
                # over sync/scalar/sync — all on scalar they serialized
                # 3 x 0.64us at the very end of the kernel
                store_engine = nc.scalar if pair == 11 else nc.sync
                store_engine.dma_start(out=out[:, q0 : q0 + 2 * SUB, :], in_=zo)

        for p in range(NPAIR):
            emit_pair_front(p)
            if p == NPAIR - 1:
                # hoist the half-block's MM1 ahead of MM2(p10): its relu
                # then completes well before MM2(tail) needs it
                emit_tail_front()
            if p >= 1:
                emit_back_pair(p - 1)
                g = _GROUP_OF[2 * (p - 1)]
                if 2 * (p - 1) + 1 == GROUPS[g][1] - 1:
                    emit_group_tail(g)
        emit_back_pair(NPAIR - 1)
        emit_group_tail(_GROUP_OF[2 * (NPAIR - 1)])
        emit_back_tail()
        emit_group_tail(_GROUP_OF[NBLK - 1])

    nc.compile()
    return nc


_NC = None


def _get_nc():
    global _NC
    if _NC is None:
        _NC = build_nc()
    return _NC


def make_in_maps(x, W1, b1, W2, b2):
    x = np.asarray(x, dtype=np.float32)
    # W1 [512, 256] -> [p, k, hid] (x16); W2 [256, 50] -> [p, kh, C] (x4)
    W1p = np.ascontiguousarray(
        (np.asarray(W1, dtype=np.float32) * W1_SCALE)
        .astype(np_fp8)
        .reshape(KC, 128, HID)
        .transpose(1, 0, 2)
    )
    W2p = np.ascontiguousarray(
        (np.asarray(W2, dtype=np.float32) * W2_SCALE)
        .astype(np_fp8)
        .reshape(MH, 128, C)
        .transpose(1, 0, 2)
    )
    # biases packed [p, MH + SUB*C]: 16*b1 columns then b2 tiled
    b1t = (np.asarray(b1, dtype=np.float32) * W1_SCALE).reshape(MH, 128).T
    b2t = np.tile(np.asarray(b2, dtype=np.float32), (128, SUB))
    bc = np.ascontiguousarray(np.concatenate([b1t, b2t], axis=1))

    in_maps = []
    full = NPAIR * 2 * BLOCK
    for i in range(N_CORES):
        r0 = i * ROWS_PER_CORE
        r1 = min(r0 + ROWS_PER_CORE, N_NODES)
        shard = np.zeros((ROWS_PER_CORE, F_IN), dtype=np_fp8)
        shard[: r1 - r0] = x[r0:r1].astype(np_fp8)
        # [rows, feat] -> [pair, p, sub, k, r] (contiguous per partition)
        xt = np.ascontiguousarray(
            shard[:full]
            .reshape(NPAIR, 2, BLOCK, KC, 128)
            .transpose(0, 4, 1, 3, 2)
        )
        # tail half block: [r, feat] -> [p, k, r]
        xtt = np.ascontiguousarray(
            shard[full:].reshape(TAIL_ROWS, KC, 128).transpose(2, 1, 0)
        )
        in_maps.append({"xT": xt, "xTt": xtt, "W1p": W1p, "W2p": W2p, "bc": bc})
    return in_maps


def run(x, W1, b1, W2, b2, trace=False, **spmd_kwargs):
    nc = _get_nc()
    in_maps = make_in_maps(x, W1, b1, W2, b2)
    res = run_bass_kernel_spmd(
        nc, in_maps, core_ids=list(range(N_CORES)), trace=trace, **spmd_kwargs
    )
    outs = []
    for i in range(N_CORES):
        o = np.asarray(res.results[i]["out"])  # [128, 98, 50] bf16, row = q*128+p
        outs.append(o.transpose(1, 0, 2).reshape(ROWS_PER_CORE, C))
    full = np.concatenate(outs, axis=0)[:N_NODES]
    return np.ascontiguousarray(full.astype(np.float32)), res


def kernel(x, edge_index, W1, b1, W2, b2):
    out, _ = run(x, W1, b1, W2, b2, trace=False)
    return out


# revision 45
# speedup vs baseline: 1.0273x; 1.0273x over previous
"""APPNP_Net Trainium2 kernel (8 NeuronCores, SPMD row-sharded), fp8 edition.

The reference model is:
    h = relu(x @ W1 + b1); z = h @ W2 + b2; out = log_softmax(z, axis=1)
followed by K=10 APPNP propagation steps with ALPHA=1.0.  Since
z_{t+1} = (1-ALPHA)*agg + ALPHA*h == h, the propagation is the identity
and edge_index never affects the output.  So the kernel is a row-wise
MLP + log_softmax, sharded by nodes across the 8 cores.

v4 changes (52.7us -> target ~46us), from NTFF trace analysis of v3:
  - x is packed block-major on the host ([pair, sub, p, k, r]), so every
    per-block DMA slice moves 1-2KB contiguous partition lines instead
    of the 512B strided runs that made block0's k01 land at 11.2us.
  - head descriptors split across both HWDGE queues: scalar writes
    block0-k01's descriptor (its first op, before the ACT_TABLE_LOAD),
    sync writes W1 / block0-k23 / biases / block1 / pair1.  ~2us earlier
    first matmul.
  - 10 warm-up matmuls (16 overshot: the last ones delayed the real
    stream; HAM's SHORT window kept resetting across the data-wait gap).
  - 12 x bufs (6 pairs in flight): v3's 1.9+0.9us PE gaps at 41-44us
    were the DMA rate-matching the shallow ring, then falling behind.
  - z-chain fused per PAIR: both blocks' MM2 outputs land in ONE psum
    bank ([128,2,4,50] = 1.6KB), so scale+bias / exp / row-sum /
    broadcast-subtract each run once per pair on 2x the elements —
    per-op overhead (~0.1-0.3us each on ACT/DVE/GpSimd) halves.
    ACT busy 31.4 -> ~26, DVE 31.5 -> ~24 predicted.

Numerics (unchanged): fp8e4 DoubleRow MM1 with W1'=16*W1, b1'=16*b1;
MM2 is plain fp8 (FD=50 < 128 makes DoubleRow's interleaved LDWEIGHTS a
net loss; FWL loads are ~50ns) with W2'=4*W2 so pz=64*(z-b2); t = pz/64
+ b2 fused in one DVE scalar_tensor_tensor; ACT table pinned; x fp8,
outputs bf16.  Rel err ~3.7e-3 vs the 2e-2 gate.
"""

import sys

sys.path.insert(0, "/opt/trn_rl_repo")

import dataclasses
from contextlib import ExitStack

import numpy as np
import ml_dtypes

import bass_rust as _bass_rust

import concourse.tile as tile
from concourse import bacc, mybir
from concourse.bass_utils import run_bass_kernel_spmd
from concourse.hw_specs import get_activation_tables

# NOTE on two dead ends, for future sessions: (1) walrus
# --enable-ldw-opt=true (redundant-LDWEIGHTS dedupe) rejects bass kernels
# (bass always emits standalone InstLdweights).  (2) capping walrus
# --max-sem-num to shrink the ~8.1us end-of-kernel semaphore-clear epilogue
# REGRESSES ~9us: fewer semaphores serialize the DMA queues, and the
# epilogue (a fixed 253-clear sweep split across the 5 engines) doesn't
# shrink.  The epilogue is ucode-fixed overhead inside the measured window.

N_NODES = 100000
F_IN = 512
HID = 256
C = 50
N_CORES = 8
BLOCK = 512
NBLK = 25  # 24 full blocks + 1 half block
NPAIR = 12  # full-block pairs
ROWS_PER_CORE = 12544  # 98 * 128; 8 * 12544 = 100352 >= 100000 (zero-padded)
TAIL_ROWS = ROWS_PER_CORE - NPAIR * 2 * BLOCK  # 256
# log-softmax groups (in blocks; boundaries must be even so pairs don't
# straddle groups): late groups are small so their serial ln/subtract
# chains overlap the remaining PE work instead of trailing it
GROUPS = [(0, 8), (8, 16), (16, 20), (20, 24), (24, 25)]
KC = F_IN // 128  # 4 contraction chunks for MM1
MH = HID // 128  # 2 hidden chunks
SUB = BLOCK // 128  # 4 row-subtiles per full block
NSUB = ROWS_PER_CORE // 128  # 98
N_WARM = 10  # dummy PE matmuls riding out the head DMA (~1.3us)

W1_SCALE = 16.0  # W1, b1 pre-scaled by this on host (fp8 normal range)
W2_SCALE = 4.0  # W2 pre-scaled by this on host
PZ_INV = 1.0 / (W1_SCALE * W2_SCALE)  # pz = 64*(z-b2); t = pz*PZ_INV + b2

BF16 = mybir.dt.bfloat16
F32 = mybir.dt.float32
FP8 = mybir.dt.float8e4
DR = mybir.MatmulPerfMode.DoubleRow
np_bf16 = ml_dtypes.bfloat16
np_fp8 = ml_dtypes.float8_e4m3

_GROUP_OF = {}
for _gi, (_s, _e) in enumerate(GROUPS):
    for _b in range(_s, _e):
        _GROUP_OF[_b] = _gi


def _bcast_cols(ap2d, reps):
    """[P, Q] AP -> [P, Q, reps] AP with a zero-stride inner dim."""
    return dataclasses.replace(ap2d, ap=[ap2d.ap[0], ap2d.ap[1], [0, reps]])


def _rep_dim(ap3d, reps):
    """[P, A, B] AP -> [P, reps, A, B] AP with a zero-stride leading dim."""
    return dataclasses.replace(
        ap3d, ap=[ap3d.ap[0], [0, reps], ap3d.ap[1], ap3d.ap[2]]
    )


def _pin_act_table(nc):
    """Constrain the ACT-table placement pass to natural_log_exp_and_others,
    which serves Relu, Exp AND Ln at full (400-bucket) resolution.  The
    default fixpoint picks exp_and_others for the steady Relu/Exp loop and
    switches tables around every Ln, costing ~1.3us per ACT_TABLE_LOAD; one
    table means one load.  Indices into the full list are preserved (the
    emitted act_func_set_id indexes act_info.json's act_func_sets)."""

    def patched():
        tables = [
            (name, funcs if name == "natural_log_exp_and_others" else set())
            for name, funcs in get_activation_tables(nc.m.arch).items()
        ]
        _bass_rust.insert_act_table_loads(nc, tables)

    nc.insert_act_table_loads = patched


def build_nc():
    nc = bacc.Bacc(
        "TRN2",
        target_bir_lowering=False,
        debug=False,
        num_devices=N_CORES,
    )
    _pin_act_table(nc)
    # [pair, p, sub, k, r]: a whole-pair transfer is 4KB contiguous per
    # partition line, per-block head slices are 1-2KB contiguous, and no
    # permuted DMA access patterns are needed anywhere
    xT = nc.declare_dram_parameter(
        "xT", [NPAIR, 128, 2, KC, BLOCK], FP8, isOutput=False
    ).ap()
    xTt = nc.declare_dram_parameter(
        "xTt", [128, KC, TAIL_ROWS], FP8, isOutput=False
    ).ap()
    # W1 packed [p, k, hid] (x16); W2 packed [p, kh, C] (x4); biases
    # packed [p, MH + SUB*C] (16*b1 columns then b2 broadcast)
    W1p = nc.declare_dram_parameter("W1p", [128, KC, HID], FP8, isOutput=False).ap()
    W2p = nc.declare_dram_parameter("W2p", [128, MH, C], FP8, isOutput=False).ap()
    bc = nc.declare_dram_parameter("bc", [128, MH + SUB * C], F32, isOutput=False).ap()
    # out[p, q, c] with row = q*128 + p (host transposes back)
    out = nc.declare_dram_parameter("out", [128, NSUB, C], BF16, isOutput=True).ap()

    with tile.TileContext(nc) as tc, ExitStack() as ctx:
        consts = ctx.enter_context(tc.tile_pool(name="consts", bufs=1))
        # 13 x bufs: pairs 0-11 + the tail each get a slot, so every x
        # descriptor can be written before the first out-store descriptor.
        # Store descriptors carry semaphore WAITS and block the sync queue
        # until their subtract fires — pair 11's x descriptor emitted after
        # them executed only at ~31us, starving the PE at ~36us for 1.7us.
        xpool = ctx.enter_context(tc.tile_pool(name="x", bufs=13))
        hpool = ctx.enter_context(tc.tile_pool(name="h", bufs=5))
        tpool = ctx.enter_context(tc.tile_pool(name="t", bufs=3))
        epool = ctx.enter_context(tc.tile_pool(name="e", bufs=3))
        spool = ctx.enter_context(tc.tile_pool(name="s", bufs=2))
        opool = ctx.enter_context(tc.tile_pool(name="o", bufs=4))
        # ph0/ph1 rings hold MM1 accumulators (6 banks); the pz ring holds
        # MM2 pair-outputs (1 bank each: 2*4*50 fp32 = 1.6KB) + warm-up
        psum = ctx.enter_context(tc.tile_pool(name="psum", bufs=3, space="PSUM"))
        psumz = ctx.enter_context(tc.tile_pool(name="psumz", bufs=2, space="PSUM"))

        # ── PE warm-up ────────────────────────────────────────────────
        # HAM throttles an idle PE to 1.2 GHz and needs ~3.4us of sustained
        # matmul activity to lift.  Burn the head's DMA wait with dummy DR
        # matmuls on a memset scratch so the real MM1 stream starts warm-ish.
        warm = consts.tile([128, 2, 128], FP8, tag="warm")
        nc.vector.memset(warm, 0.25)
        wps = psumz.tile([128, 128], F32, tag="pz", name="warm_ps")
        for _ in range(N_WARM):
            nc.tensor.matmul(
                wps, lhsT=warm, rhs=warm, start=True, stop=True, perf_mode=DR
            )

        # ── head DMAs, split across both HWDGE queues ─────────────────
        # DIRECT2D descriptor writes cost ~0.65us each and serialize per
        # engine.  scalar's first op is block0-k01's descriptor (the first
        # matmul's gating data, in flight by ~8us); sync carries W1 then
        # the rest in need-order.  gpsimd's SWDGE queue stays unused.
        # sync carries the x stream in need-order (k01, k23, block1,
        # pair1, ...) so every chunk's descriptor lands ~0.65us earlier
        # than with W1 in front; scalar carries the small consts (W1
        # 128KB, biases, W2) whose transfers finish almost instantly.
        xts = {}
        xt0 = xpool.tile([128, 2, KC, BLOCK], FP8, tag="xt", name="xt0")
        nc.sync.dma_start(out=xt0[:, 0, :2, :], in_=xT[0, :, 0, :2, :])
        w1t = consts.tile([128, KC, HID], FP8, tag="w1")
        nc.scalar.dma_start(out=w1t, in_=W1p)
        nc.sync.dma_start(out=xt0[:, 0, 2:, :], in_=xT[0, :, 0, 2:, :])
        bct = consts.tile([128, MH + SUB * C], F32, tag="bc")
        nc.scalar.dma_start(out=bct, in_=bc)
        nc.sync.dma_start(out=xt0[:, 1], in_=xT[0, :, 1])
        b1sb = bct[:, :MH]
        b2sb = bct[:, MH:].rearrange("p (s c) -> p s c", s=SUB)
        w2t = consts.tile([128, MH, C], FP8, tag="w2")
        nc.scalar.dma_start(out=w2t, in_=W2p)
        xts[0] = xt0
        xt1 = xpool.tile([128, 2, KC, BLOCK], FP8, tag="xt", name="xt1")
        nc.sync.dma_start(out=xt1, in_=xT[1])
        xts[1] = xt1

        def issue_pair(p):
            if p == NPAIR:
                xt = xpool.tile([128, KC, TAIL_ROWS], FP8, tag="xt", name="xt_tail")
                nc.sync.dma_start(out=xt, in_=xTt)
            else:
                xt = xpool.tile([128, 2, KC, BLOCK], FP8, tag="xt", name=f"xt{p}")
                nc.sync.dma_start(out=xt, in_=xT[p])
            xts[p] = xt

        hs = {}  # block -> h tile [128, MH, BLOCK] fp8 (16*relu(...))
        t_gs = {}  # group -> t tile
        s_gs = {}  # group -> s tile

        # With the ACT table pinned (no Ln switches), relu routing is pure
        # load balance: ACT takes mh0 always, plus mh1 on a few blocks so
        # ACT (~relu0+exp+ln) and DVE (~relu1+scale-bias+reduce) even out.
        # block 23's relu on ACT relieves DVE right where MM2(p11)/MM2(tail)
        # were stalling ~1.3us on relu semaphores at the end of the stream
        act_mh1 = {5, 9, 13, 23}

        def emit_relu(b, mh, ph, R):
            ht = hs[b]
            if mh == 0 or b in act_mh1:
                # ScalarE: 16h = relu(ph + 16*b1)
                nc.scalar.activation(
                    ht[:, mh, :R],
                    ph[:, :R],
                    mybir.ActivationFunctionType.Relu,
                    bias=b1sb[:, mh : mh + 1],
                )
            else:
                # VectorE: (ph + 16*b1) max 0 — balance the engines
                nc.vector.tensor_scalar(
                    out=ht[:, mh, :R],
                    in0=ph[:, :R],
                    scalar1=b1sb[:, mh : mh + 1],
                    scalar2=0.0,
                    op0=mybir.AluOpType.add,
                    op1=mybir.AluOpType.max,
                )

        def emit_pair_front(p):
            """x prefetch, MM1 (fp8 DR, mh->k2->block), relu for pair p."""
            for pf in range(2, min(p + 8, NPAIR + 1)):
                if pf not in xts:
                    issue_pair(pf)
            xt = xts[p]
            blocks = [2 * p, 2 * p + 1]
            for b in blocks:
                hs[b] = hpool.tile([128, MH, BLOCK], FP8, tag="h", name=f"h{b}")
            for mh in range(MH):
                phs = [
                    psum.tile([128, BLOCK], F32, tag=f"ph{mh}", name=f"ph{mh}_{b}")
                    for b in blocks
                ]
                for k2 in range(KC // 2):
                    for s in range(2):
                        nc.tensor.matmul(
                            phs[s],
                            lhsT=w1t[:, 2 * k2 : 2 * k2 + 2, mh * 128 : (mh + 1) * 128],
                            rhs=xt[:, s, 2 * k2 : 2 * k2 + 2, :],
                            start=(k2 == 0),
                            stop=(k2 == KC // 2 - 1),
                            perf_mode=DR,
                        )
                for s, b in enumerate(blocks):
                    emit_relu(b, mh, phs[s], BLOCK)
                if p < 2:
                    # bridge the early x-DMA gaps (pair1/2 land ~12-14us)
                    # with dependency-free dummies so HAM's 3.4us
                    # continuous-activity window isn't reset — each reset
                    # costs ~1.5-3us of half-clock PE time; a dummy costs
                    # ~107ns only if the real data was late anyway
                    for _ in range(3):
                        nc.tensor.matmul(
                            wps, lhsT=warm, rhs=warm,
                            start=True, stop=True, perf_mode=DR,
                        )

        def emit_tail_front():
            """MM1 + relu for the final half block (256 rows)."""
            b = NBLK - 1
            R = TAIL_ROWS
            if NPAIR not in xts:
                issue_pair(NPAIR)
            xt = xts[NPAIR]
            hs[b] = hpool.tile([128, MH, BLOCK], FP8, tag="h", name=f"h{b}")
            for mh in range(MH):
                ph = psum.tile([128, BLOCK], F32, tag=f"ph{mh}", name=f"ph{mh}_t")
                for k2 in range(KC // 2):
                    nc.tensor.matmul(
                        ph[:, :R],
                        lhsT=w1t[:, 2 * k2 : 2 * k2 + 2, mh * 128 : (mh + 1) * 128],
                        rhs=xt[:, 2 * k2 : 2 * k2 + 2, :R],
                        start=(k2 == 0),
                        stop=(k2 == KC // 2 - 1),
                        perf_mode=DR,
                    )
                emit_relu(b, mh, ph, R)

        def _group_tiles(g):
            g0, g1 = GROUPS[g]
            glen = g1 - g0
            t_gs[g] = tpool.tile([128, glen, SUB, C], BF16, tag="t", name=f"t_g{g}")
            s_gs[g] = spool.tile([128, glen * SUB], BF16, tag="s", name=f"s_g{g}")

        def emit_back_pair(p):
            """MM2 (both blocks into ONE psum bank), then pair-granular
            scale+bias, exp, row-sum.  Halves the per-op overhead vs
            per-block ops."""
            b0 = 2 * p
            g = _GROUP_OF[b0]
            g0, g1 = GROUPS[g]
            j = b0 - g0
            if j == 0:
                _group_tiles(g)
            t_g, s_g = t_gs[g], s_gs[g]
            pz = psumz.tile([128, 2, SUB, C], F32, tag="pz", name=f"pz{p}")
            for s in range(2):
                ht = hs.pop(b0 + s)
                # MM2 runs non-DR: at FD=50 DoubleRow's interleaved
                # LDWEIGHTS (~140ns) loses to FWL (~50ns); two accumulating
                # 128-contraction matmuls per subtile
                for rs in range(SUB):
                    for mh in range(MH):
                        nc.tensor.matmul(
                            pz[:, s, rs, :],
                            lhsT=ht[:, mh, rs * 128 : (rs + 1) * 128],
                            rhs=w2t[:, mh, :],
                            start=(mh == 0),
                            stop=(mh == MH - 1),
                        )
            # t = z = pz/64 + b2 for both blocks in one DVE op
            nc.vector.scalar_tensor_tensor(
                out=t_g[:, j : j + 2],
                in0=pz,
                scalar=PZ_INV,
                in1=_rep_dim(b2sb, 2),
                op0=mybir.AluOpType.mult,
                op1=mybir.AluOpType.add,
            )
            e = epool.tile([128, 2, SUB, C], BF16, tag="e", name=f"e{p}")
            nc.scalar.activation(
                e, t_g[:, j : j + 2], mybir.ActivationFunctionType.Exp
            )
            with nc.allow_low_precision(reason="bf16 softmax row-sum"):
                nc.vector.reduce_sum(
                    out=s_g[:, j * SUB : (j + 2) * SUB].rearrange(
                        "p (a s) -> p a s", a=2
                    ),
                    in_=e,
                    axis=mybir.AxisListType.X,
                )

        def emit_back_tail():
            """MM2 + z-chain for the final half block."""
            b = NBLK - 1
            S = TAIL_ROWS // 128
            g = _GROUP_OF[b]
            g0, _ = GROUPS[g]
            j = b - g0
            if j == 0:
                _group_tiles(g)
            t_g, s_g = t_gs[g], s_gs[g]
            ht = hs.pop(b)
            pz = psumz.tile([128, SUB, C], F32, tag="pz", name="pz_t")
            for rs in range(S):
                for mh in range(MH):
                    nc.tensor.matmul(
                        pz[:, rs, :],
                        lhsT=ht[:, mh, rs * 128 : (rs + 1) * 128],
                        rhs=w2t[:, mh, :],
                        start=(mh == 0),
                        stop=(mh == MH - 1),
                    )
            nc.vector.scalar_tensor_tensor(
                out=t_g[:, j, :S, :],
                in0=pz[:, :S, :],
                scalar=PZ_INV,
                in1=b2sb[:, :S, :],
                op0=mybir.AluOpType.mult,
                op1=mybir.AluOpType.add,
            )
            e = epool.tile([128, SUB, C], BF16, tag="e", name="e_t")
            nc.scalar.activation(
                e[:, :S, :], t_g[:, j, :S, :], mybir.ActivationFunctionType.Exp
            )
            with nc.allow_low_precision(reason="bf16 softmax row-sum"):
                nc.vector.reduce_sum(
                    out=s_g[:, j * SUB : j * SUB + S],
                    in_=e[:, :S, :],
                    axis=mybir.AxisListType.X,
                )

        def emit_group_tail(g):
            """ls = ln(s); out = z - ls (class-broadcast) per pair; store."""
            g0, g1 = GROUPS[g]
            t_g, s_g = t_gs.pop(g), s_gs.pop(g)
            last = g1 == NBLK
            ncols = (g1 - g0 - 1) * SUB + (TAIL_ROWS // 128 if last else SUB)
            ls_g = spool.tile(
                [128, (g1 - g0) * SUB], BF16, tag="ls", name=f"ls_g{g}"
            )
            nc.scalar.activation(
                ls_g[:, :ncols], s_g[:, :ncols], mybir.ActivationFunctionType.Ln
            )
            for j in range(0, g1 - g0, 2):
                b = g0 + j
                pair = b // 2
                if b == NBLK - 1:  # half-block tail, alone in its group
                    S = TAIL_ROWS // 128
                    zo = opool.tile([128, S, C], BF16, tag="zo", name="zo_t")
                    nc.vector.tensor_tensor(
                        out=zo,
                        in0=t_g[:, j, :S, :],
                        in1=_bcast_cols(ls_g[:, j * SUB : j * SUB + S], C),
                        op=mybir.AluOpType.subtract,
                    )
                    # sync is idle by now; scalar carries pair 11's store
                    # scalar is idle after its last ln; the tail store on
                    # sync was serializing behind pair 10's descriptor
                    nc.scalar.dma_start(
                        out=out[:, pair * 2 * SUB : pair * 2 * SUB + S, :], in_=zo
                    )
                    continue
                zo = opool.tile([128, 2 * SUB, C], BF16, tag="zo", name=f"zo{pair}")
                if g == len(GROUPS) - 2 and j >= g1 - g0 - 2:
                    sub_engine = nc.vector  # last full pair: DVE is freeing up
                else:
                    sub_engine = nc.gpsimd  # mid-stream: keep DVE for relu
                sub_engine.tensor_tensor(
                    out=zo,
                    in0=t_g[:, j : j + 2].rearrange("p a s c -> p (a s) c"),
                    in1=_bcast_cols(ls_g[:, j * SUB : (j + 2) * SUB], C),
                    op=mybir.AluOpType.subtract,
                )
                q0 = pair * 2 * SUB
                # spread the three final store descriptors (p10, p11, tail)
                # over sync/scalar/sync — all on scalar they serialized
                # 3 x 0.64us at the very end of the kernel
                store_engine = nc.scalar if pair == 11 else nc.sync
                store_engine.dma_start(out=out[:, q0 : q0 + 2 * SUB, :], in_=zo)

        for p in range(NPAIR):
            emit_pair_front(p)
            if p == NPAIR - 2:
                # hoist the half-block's MM1 ahead of MM2(p10): its relu
                # then completes well before MM2(tail) needs it
                emit_tail_front()
            if p >= 1:
                emit_back_pair(p - 1)
                g = _GROUP_OF[2 * (p - 1)]
                if 2 * (p - 1) + 1 == GROUPS[g][1] - 1:
                    emit_group_tail(g)
        emit_back_pair(NPAIR - 1)
        emit_group_tail(_GROUP_OF[2 * (NPAIR - 1)])
        emit_back_tail()
        emit_group_tail(_GROUP_OF[NBLK - 1])

    nc.compile()
    return nc


_NC = None


def _get_nc():
    global _NC
    if _NC is None:
        _NC = build_nc()
    return _NC


def make_in_maps(x, W1, b1, W2, b2):
    x = np.asarray(x, dtype=np.float32)
    # W1 [512, 256] -> [p, k, hid] (x16); W2 [256, 50] -> [p, kh, C] (x4)
    W1p = np.ascontiguousarray(
        (np.asarray(W1, dtype=np.float32) * W1_SCALE)
        .astype(np_fp8)
        .reshape(KC, 128, HID)
        .transpose(1, 0, 2)
    )
    W2p = np.ascontiguousarray(
        (np.asarray(W2, dtype=np.float32) * W2_SCALE)
        .astype(np_fp8)
        .reshape(MH, 128, C)
        .transpose(1, 0, 2)
    )
    # biases packed [p, MH + SUB*C]: 16*b1 columns then b2 tiled
    b1t = (np.asarray(b1, dtype=np.float32) * W1_SCALE).reshape(MH, 128).T
    b2t = np.tile(np.asarray(b2, dtype=np.float32), (128, SUB))
    bc = np.ascontiguousarray(np.concatenate([b1t, b2t], axis=1))

    in_maps = []
    full = NPAIR * 2 * BLOCK
    for i in range(N_CORES):
        r0 = i * ROWS_PER_CORE
        r1 = min(r0 + ROWS_PER_CORE, N_NODES)
        shard = np.zeros((ROWS_PER_CORE, F_IN), dtype=np_fp8)
        shard[: r1 - r0] = x[r0:r1].astype(np_fp8)
        # [rows, feat] -> [pair, p, sub, k, r] (contiguous per partition)
        xt = np.ascontiguousarray(
            shard[:full]
            .reshape(NPAIR, 2, BLOCK, KC, 128)
            .transpose(0, 4, 1, 3, 2)
        )
        # tail half block: [r, feat] -> [p, k, r]
        xtt = np.ascontiguousarray(
            shard[full:].reshape(TAIL_ROWS, KC, 128).transpose(2, 1, 0)
        )
        in_maps.append({"xT": xt, "xTt": xtt, "W1p": W1p, "W2p": W2p, "bc": bc})
    return in_maps


def run(x, W1, b1, W2, b2, trace=False, **spmd_kwargs):
    nc = _get_nc()
    in_maps = make_in_maps(x, W1, b1, W2, b2)
    res = run_bass_kernel_spmd(
        nc, in_maps, core_ids=list(range(N_CORES)), trace=trace, **spmd_kwargs
    )
    outs = []
    for i in range(N_CORES):
        o = np.asarray(res.results[i]["out"])  # [128, 98, 50] bf16, row = q*128+p
        outs.append(o.transpose(1, 0, 2).reshape(ROWS_PER_CORE, C))
    full = np.concatenate(outs, axis=0)[:N_NODES]
    return np.ascontiguousarray(full.astype(np.float32)), res


def kernel(x, edge_index, W1, b1, W2, b2):
    out, _ = run(x, W1, b1, W2, b2, trace=False)
    return out


# revision 51
# speedup vs baseline: 1.0348x; 1.0074x over previous
"""APPNP_Net Trainium2 kernel (8 NeuronCores, SPMD row-sharded), fp8 edition.

The reference model is:
    h = relu(x @ W1 + b1); z = h @ W2 + b2; out = log_softmax(z, axis=1)
followed by K=10 APPNP propagation steps with ALPHA=1.0.  Since
z_{t+1} = (1-ALPHA)*agg + ALPHA*h == h, the propagation is the identity
and edge_index never affects the output.  So the kernel is a row-wise
MLP + log_softmax, sharded by nodes across the 8 cores.

v4 changes (52.7us -> target ~46us), from NTFF trace analysis of v3:
  - x is packed block-major on the host ([pair, sub, p, k, r]), so every
    per-block DMA slice moves 1-2KB contiguous partition lines instead
    of the 512B strided runs that made block0's k01 land at 11.2us.
  - head descriptors split across both HWDGE queues: scalar writes
    block0-k01's descriptor (its first op, before the ACT_TABLE_LOAD),
    sync writes W1 / block0-k23 / biases / block1 / pair1.  ~2us earlier
    first matmul.
  - 10 warm-up matmuls (16 overshot: the last ones delayed the real
    stream; HAM's SHORT window kept resetting across the data-wait gap).
  - 12 x bufs (6 pairs in flight): v3's 1.9+0.9us PE gaps at 41-44us
    were the DMA rate-matching the shallow ring, then falling behind.
  - z-chain fused per PAIR: both blocks' MM2 outputs land in ONE psum
    bank ([128,2,4,50] = 1.6KB), so scale+bias / exp / row-sum /
    broadcast-subtract each run once per pair on 2x the elements —
    per-op overhead (~0.1-0.3us each on ACT/DVE/GpSimd) halves.
    ACT busy 31.4 -> ~26, DVE 31.5 -> ~24 predicted.

Numerics (unchanged): fp8e4 DoubleRow MM1 with W1'=16*W1, b1'=16*b1;
MM2 is plain fp8 (FD=50 < 128 makes DoubleRow's interleaved LDWEIGHTS a
net loss; FWL loads are ~50ns) with W2'=4*W2 so pz=64*(z-b2); t = pz/64
+ b2 fused in one DVE scalar_tensor_tensor; ACT table pinned; x fp8,
outputs bf16.  Rel err ~3.7e-3 vs the 2e-2 gate.
"""

import sys

sys.path.insert(0, "/opt/trn_rl_repo")

import dataclasses
from contextlib import ExitStack

import numpy as np
import ml_dtypes

import bass_rust as _bass_rust

import concourse.tile as tile
from concourse import bacc, mybir
from concourse.bass_utils import run_bass_kernel_spmd
from concourse.hw_specs import get_activation_tables

# NOTE on two dead ends, for future sessions: (1) walrus
# --enable-ldw-opt=true (redundant-LDWEIGHTS dedupe) rejects bass kernels
# (bass always emits standalone InstLdweights).  (2) capping walrus
# --max-sem-num to shrink the ~8.1us end-of-kernel semaphore-clear epilogue
# REGRESSES ~9us: fewer semaphores serialize the DMA queues, and the
# epilogue (a fixed 253-clear sweep split across the 5 engines) doesn't
# shrink.  The epilogue is ucode-fixed overhead inside the measured window.

N_NODES = 100000
F_IN = 512
HID = 256
C = 50
N_CORES = 8
BLOCK = 512
NBLK = 25  # 24 full blocks + 1 half block
NPAIR = 12  # full-block pairs
ROWS_PER_CORE = 12544  # 98 * 128; 8 * 12544 = 100352 >= 100000 (zero-padded)
TAIL_ROWS = ROWS_PER_CORE - NPAIR * 2 * BLOCK  # 256
# log-softmax groups (in blocks; boundaries must be even so pairs don't
# straddle groups): late groups are small so their serial ln/subtract
# chains overlap the remaining PE work instead of trailing it
GROUPS = [(0, 8), (8, 16), (16, 20), (20, 24), (24, 25)]
KC = F_IN // 128  # 4 contraction chunks for MM1
MH = HID // 128  # 2 hidden chunks
SUB = BLOCK // 128  # 4 row-subtiles per full block
NSUB = ROWS_PER_CORE // 128  # 98
N_WARM = 10  # dummy PE matmuls riding out the head DMA (~1.3us)

W1_SCALE = 16.0  # W1, b1 pre-scaled by this on host (fp8 normal range)
W2_SCALE = 4.0  # W2 pre-scaled by this on host
PZ_INV = 1.0 / (W1_SCALE * W2_SCALE)  # pz = 64*(z-b2); t = pz*PZ_INV + b2

BF16 = mybir.dt.bfloat16
F32 = mybir.dt.float32
FP8 = mybir.dt.float8e4
DR = mybir.MatmulPerfMode.DoubleRow
np_bf16 = ml_dtypes.bfloat16
np_fp8 = ml_dtypes.float8_e4m3

_GROUP_OF = {}
for _gi, (_s, _e) in enumerate(GROUPS):
    for _b in range(_s, _e):
        _GROUP_OF[_b] = _gi


def _bcast_cols(ap2d, reps):
    """[P, Q] AP -> [P, Q, reps] AP with a zero-stride inner dim."""
    return dataclasses.replace(ap2d, ap=[ap2d.ap[0], ap2d.ap[1], [0, reps]])


def _rep_dim(ap3d, reps):
    """[P, A, B] AP -> [P, reps, A, B] AP with a zero-stride leading dim."""
    return dataclasses.replace(
        ap3d, ap=[ap3d.ap[0], [0, reps], ap3d.ap[1], ap3d.ap[2]]
    )


def _pin_act_table(nc):
    """Constrain the ACT-table placement pass to natural_log_exp_and_others,
    which serves Relu, Exp AND Ln at full (400-bucket) resolution.  The
    default fixpoint picks exp_and_others for the steady Relu/Exp loop and
    switches tables around every Ln, costing ~1.3us per ACT_TABLE_LOAD; one
    table means one load.  Indices into the full list are preserved (the
    emitted act_func_set_id indexes act_info.json's act_func_sets)."""

    def patched():
        tables = [
            (name, funcs if name == "natural_log_exp_and_others" else set())
            for name, funcs in get_activation_tables(nc.m.arch).items()
        ]
        _bass_rust.insert_act_table_loads(nc, tables)

    nc.insert_act_table_loads = patched


def build_nc():
    nc = bacc.Bacc(
        "TRN2",
        target_bir_lowering=False,
        debug=False,
        num_devices=N_CORES,
    )
    _pin_act_table(nc)
    # [pair, p, sub, k, r]: a whole-pair transfer is 4KB contiguous per
    # partition line, per-block head slices are 1-2KB contiguous, and no
    # permuted DMA access patterns are needed anywhere
    xT = nc.declare_dram_parameter(
        "xT", [NPAIR, 128, 2, KC, BLOCK], FP8, isOutput=False
    ).ap()
    xTt = nc.declare_dram_parameter(
        "xTt", [128, KC, TAIL_ROWS], FP8, isOutput=False
    ).ap()
    # W1 packed [p, k, hid] (x16); W2 packed [p, kh, C] (x4); biases
    # packed [p, MH + SUB*C] (16*b1 columns then b2 broadcast)
    W1p = nc.declare_dram_parameter("W1p", [128, KC, HID], FP8, isOutput=False).ap()
    W2p = nc.declare_dram_parameter("W2p", [128, MH, C], FP8, isOutput=False).ap()
    bc = nc.declare_dram_parameter("bc", [128, MH + SUB * C], F32, isOutput=False).ap()
    # out[p, q, c] with row = q*128 + p (host transposes back)
    out = nc.declare_dram_parameter("out", [128, NSUB, C], BF16, isOutput=True).ap()

    with tile.TileContext(nc) as tc, ExitStack() as ctx:
        consts = ctx.enter_context(tc.tile_pool(name="consts", bufs=1))
        # 13 x bufs: pairs 0-11 + the tail each get a slot, so every x
        # descriptor can be written before the first out-store descriptor.
        # Store descriptors carry semaphore WAITS and block the sync queue
        # until their subtract fires — pair 11's x descriptor emitted after
        # them executed only at ~31us, starving the PE at ~36us for 1.7us.
        xpool = ctx.enter_context(tc.tile_pool(name="x", bufs=13))
        hpool = ctx.enter_context(tc.tile_pool(name="h", bufs=5))
        tpool = ctx.enter_context(tc.tile_pool(name="t", bufs=3))
        epool = ctx.enter_context(tc.tile_pool(name="e", bufs=3))
        spool = ctx.enter_context(tc.tile_pool(name="s", bufs=2))
        opool = ctx.enter_context(tc.tile_pool(name="o", bufs=4))
        # ph0/ph1 rings hold MM1 accumulators (6 banks); the pz ring holds
        # MM2 pair-outputs (1 bank each: 2*4*50 fp32 = 1.6KB) + warm-up
        psum = ctx.enter_context(tc.tile_pool(name="psum", bufs=3, space="PSUM"))
        psumz = ctx.enter_context(tc.tile_pool(name="psumz", bufs=2, space="PSUM"))

        # ── PE warm-up ────────────────────────────────────────────────
        # HAM throttles an idle PE to 1.2 GHz and needs ~3.4us of sustained
        # matmul activity to lift.  Burn the head's DMA wait with dummy DR
        # matmuls on a memset scratch so the real MM1 stream starts warm-ish.
        warm = consts.tile([128, 2, 128], FP8, tag="warm")
        nc.vector.memset(warm, 0.25)
        wps = psumz.tile([128, 128], F32, tag="pz", name="warm_ps")
        for _ in range(N_WARM):
            nc.tensor.matmul(
                wps, lhsT=warm, rhs=warm, start=True, stop=True, perf_mode=DR
            )

        # ── head DMAs, split across both HWDGE queues ─────────────────
        # DIRECT2D descriptor writes cost ~0.65us each and serialize per
        # engine.  scalar's first op is block0-k01's descriptor (the first
        # matmul's gating data, in flight by ~8us); sync carries W1 then
        # the rest in need-order.  gpsimd's SWDGE queue stays unused.
        # sync carries the x stream in need-order (k01, k23, block1,
        # pair1, ...) so every chunk's descriptor lands ~0.65us earlier
        # than with W1 in front; scalar carries the small consts (W1
        # 128KB, biases, W2) whose transfers finish almost instantly.
        xts = {}
        xt0 = xpool.tile([128, 2, KC, BLOCK], FP8, tag="xt", name="xt0")
        nc.sync.dma_start(out=xt0[:, 0, :2, :], in_=xT[0, :, 0, :2, :])
        w1t = consts.tile([128, KC, HID], FP8, tag="w1")
        nc.scalar.dma_start(out=w1t, in_=W1p)
        nc.sync.dma_start(out=xt0[:, 0, 2:, :], in_=xT[0, :, 0, 2:, :])
        bct = consts.tile([128, MH + SUB * C], F32, tag="bc")
        nc.scalar.dma_start(out=bct, in_=bc)
        nc.sync.dma_start(out=xt0[:, 1], in_=xT[0, :, 1])
        b1sb = bct[:, :MH]
        b2sb = bct[:, MH:].rearrange("p (s c) -> p s c", s=SUB)
        w2t = consts.tile([128, MH, C], FP8, tag="w2")
        nc.scalar.dma_start(out=w2t, in_=W2p)
        xts[0] = xt0
        xt1 = xpool.tile([128, 2, KC, BLOCK], FP8, tag="xt", name="xt1")
        nc.sync.dma_start(out=xt1, in_=xT[1])
        xts[1] = xt1

        def issue_pair(p):
            if p == NPAIR:
                xt = xpool.tile([128, KC, TAIL_ROWS], FP8, tag="xt", name="xt_tail")
                nc.sync.dma_start(out=xt, in_=xTt)
            else:
                xt = xpool.tile([128, 2, KC, BLOCK], FP8, tag="xt", name=f"xt{p}")
                nc.sync.dma_start(out=xt, in_=xT[p])
            xts[p] = xt

        hs = {}  # block -> h tile [128, MH, BLOCK] fp8 (16*relu(...))
        t_gs = {}  # group -> t tile
        s_gs = {}  # group -> s tile

        # With the ACT table pinned (no Ln switches), relu routing is pure
        # load balance: ACT takes mh0 always, plus mh1 on a few blocks so
        # ACT (~relu0+exp+ln) and DVE (~relu1+scale-bias+reduce) even out.
        # block 23's relu on ACT relieves DVE right where MM2(p11)/MM2(tail)
        # were stalling ~1.3us on relu semaphores at the end of the stream
        act_mh1 = {5, 9, 13, 23}

        def emit_relu(b, mh, ph, R):
            ht = hs[b]
            if mh == 0 or b in act_mh1:
                # ScalarE: 16h = relu(ph + 16*b1)
                nc.scalar.activation(
                    ht[:, mh, :R],
                    ph[:, :R],
                    mybir.ActivationFunctionType.Relu,
                    bias=b1sb[:, mh : mh + 1],
                )
            else:
                # VectorE: (ph + 16*b1) max 0 — balance the engines
                nc.vector.tensor_scalar(
                    out=ht[:, mh, :R],
                    in0=ph[:, :R],
                    scalar1=b1sb[:, mh : mh + 1],
                    scalar2=0.0,
                    op0=mybir.AluOpType.add,
                    op1=mybir.AluOpType.max,
                )

        def emit_pair_front(p):
            """x prefetch, MM1 (fp8 DR, mh->k2->block), relu for pair p."""
            # throttled prefetch: 2 pairs per iteration early on — issuing
            # 6 descriptors at p=0 made pairs 3-7's transfers round-robin-
            # steal queue bandwidth from the urgently needed xt1/xt2 (the
            # 1.1-1.8us PE gaps at 10-16us).  By p=4 everything (incl. pair
            # 11 + tail) is issued, still ahead of the first store
            # descriptor whose semaphore wait would block the sync queue.
            hi = NPAIR + 1 if p >= 4 else 2 * p + 4
            for pf in range(2, hi):
                if pf not in xts:
                    issue_pair(pf)
            xt = xts[p]
            blocks = [2 * p, 2 * p + 1]
            for b in blocks:
                hs[b] = hpool.tile([128, MH, BLOCK], FP8, tag="h", name=f"h{b}")
            for mh in range(MH):
                phs = [
                    psum.tile([128, BLOCK], F32, tag=f"ph{mh}", name=f"ph{mh}_{b}")
                    for b in blocks
                ]
                for k2 in range(KC // 2):
                    for s in range(2):
                        nc.tensor.matmul(
                            phs[s],
                            lhsT=w1t[:, 2 * k2 : 2 * k2 + 2, mh * 128 : (mh + 1) * 128],
                            rhs=xt[:, s, 2 * k2 : 2 * k2 + 2, :],
                            start=(k2 == 0),
                            stop=(k2 == KC // 2 - 1),
                            perf_mode=DR,
                        )
                for s, b in enumerate(blocks):
                    emit_relu(b, mh, phs[s], BLOCK)
                if p < 2:
                    # bridge the early x-DMA gaps (pair1/2 land ~12-14us)
                    # with dependency-free dummies so HAM's 3.4us
                    # continuous-activity window isn't reset — each reset
                    # costs ~1.5-3us of half-clock PE time; a dummy costs
                    # ~107ns only if the real data was late anyway
                    for _ in range(3):
                        nc.tensor.matmul(
                            wps, lhsT=warm, rhs=warm,
                            start=True, stop=True, perf_mode=DR,
                        )

        def emit_tail_front():
            """MM1 + relu for the final half block (256 rows)."""
            b = NBLK - 1
            R = TAIL_ROWS
            if NPAIR not in xts:
                issue_pair(NPAIR)
            xt = xts[NPAIR]
            hs[b] = hpool.tile([128, MH, BLOCK], FP8, tag="h", name=f"h{b}")
            for mh in range(MH):
                ph = psum.tile([128, BLOCK], F32, tag=f"ph{mh}", name=f"ph{mh}_t")
                for k2 in range(KC // 2):
                    nc.tensor.matmul(
                        ph[:, :R],
                        lhsT=w1t[:, 2 * k2 : 2 * k2 + 2, mh * 128 : (mh + 1) * 128],
                        rhs=xt[:, 2 * k2 : 2 * k2 + 2, :R],
                        start=(k2 == 0),
                        stop=(k2 == KC // 2 - 1),
                        perf_mode=DR,
                    )
                emit_relu(b, mh, ph, R)

        def _group_tiles(g):
            g0, g1 = GROUPS[g]
            glen = g1 - g0
            t_gs[g] = tpool.tile([128, glen, SUB, C], BF16, tag="t", name=f"t_g{g}")
            s_gs[g] = spool.tile([128, glen * SUB], BF16, tag="s", name=f"s_g{g}")

        def emit_back_pair(p):
            """MM2 (both blocks into ONE psum bank), then pair-granular
            scale+bias, exp, row-sum.  Halves the per-op overhead vs
            per-block ops."""
            b0 = 2 * p
            g = _GROUP_OF[b0]
            g0, g1 = GROUPS[g]
            j = b0 - g0
            if j == 0:
                _group_tiles(g)
            t_g, s_g = t_gs[g], s_gs[g]
            pz = psumz.tile([128, 2, SUB, C], F32, tag="pz", name=f"pz{p}")
            for s in range(2):
                ht = hs.pop(b0 + s)
                # MM2 runs non-DR: at FD=50 DoubleRow's interleaved
                # LDWEIGHTS (~140ns) loses to FWL (~50ns); two accumulating
                # 128-contraction matmuls per subtile
                for rs in range(SUB):
                    for mh in range(MH):
                        nc.tensor.matmul(
                            pz[:, s, rs, :],
                            lhsT=ht[:, mh, rs * 128 : (rs + 1) * 128],
                            rhs=w2t[:, mh, :],
                            start=(mh == 0),
                            stop=(mh == MH - 1),
                        )
            # t = z = pz/64 + b2 for both blocks in one DVE op
            nc.vector.scalar_tensor_tensor(
                out=t_g[:, j : j + 2],
                in0=pz,
                scalar=PZ_INV,
                in1=_rep_dim(b2sb, 2),
                op0=mybir.AluOpType.mult,
                op1=mybir.AluOpType.add,
            )
            e = epool.tile([128, 2, SUB, C], BF16, tag="e", name=f"e{p}")
            nc.scalar.activation(
                e, t_g[:, j : j + 2], mybir.ActivationFunctionType.Exp
            )
            with nc.allow_low_precision(reason="bf16 softmax row-sum"):
                nc.vector.reduce_sum(
                    out=s_g[:, j * SUB : (j + 2) * SUB].rearrange(
                        "p (a s) -> p a s", a=2
                    ),
                    in_=e,
                    axis=mybir.AxisListType.X,
                )

        def emit_back_tail():
            """MM2 + z-chain for the final half block."""
            b = NBLK - 1
            S = TAIL_ROWS // 128
            g = _GROUP_OF[b]
            g0, _ = GROUPS[g]
            j = b - g0
            if j == 0:
                _group_tiles(g)
            t_g, s_g = t_gs[g], s_gs[g]
            ht = hs.pop(b)
            pz = psumz.tile([128, SUB, C], F32, tag="pz", name="pz_t")
            for rs in range(S):
                for mh in range(MH):
                    nc.tensor.matmul(
                        pz[:, rs, :],
                        lhsT=ht[:, mh, rs * 128 : (rs + 1) * 128],
                        rhs=w2t[:, mh, :],
                        start=(mh == 0),
                        stop=(mh == MH - 1),
                    )
            nc.vector.scalar_tensor_tensor(
                out=t_g[:, j, :S, :],
                in0=pz[:, :S, :],
                scalar=PZ_INV,
                in1=b2sb[:, :S, :],
                op0=mybir.AluOpType.mult,
                op1=mybir.AluOpType.add,
            )
            e = epool.tile([128, SUB, C], BF16, tag="e", name="e_t")
            nc.scalar.activation(
                e[:, :S, :], t_g[:, j, :S, :], mybir.ActivationFunctionType.Exp
            )
            with nc.allow_low_precision(reason="bf16 softmax row-sum"):
                nc.vector.reduce_sum(
                    out=s_g[:, j * SUB : j * SUB + S],
                    in_=e[:, :S, :],
                    axis=mybir.AxisListType.X,
                )

        def emit_group_tail(g):
            """ls = ln(s); out = z - ls (class-broadcast) per pair; store."""
            g0, g1 = GROUPS[g]
            t_g, s_g = t_gs.pop(g), s_gs.pop(g)
            last = g1 == NBLK
            ncols = (g1 - g0 - 1) * SUB + (TAIL_ROWS // 128 if last else SUB)
            ls_g = spool.tile(
                [128, (g1 - g0) * SUB], BF16, tag="ls", name=f"ls_g{g}"
            )
            nc.scalar.activation(
                ls_g[:, :ncols], s_g[:, :ncols], mybir.ActivationFunctionType.Ln
            )
            for j in range(0, g1 - g0, 2):
                b = g0 + j
                pair = b // 2
                if b == NBLK - 1:  # half-block tail, alone in its group
                    S = TAIL_ROWS // 128
                    zo = opool.tile([128, S, C], BF16, tag="zo", name="zo_t")
                    nc.vector.tensor_tensor(
                        out=zo,
                        in0=t_g[:, j, :S, :],
                        in1=_bcast_cols(ls_g[:, j * SUB : j * SUB + S], C),
                        op=mybir.AluOpType.subtract,
                    )
                    # sync is idle by now; scalar carries pair 11's store
                    # scalar is idle after its last ln; the tail store on
                    # sync was serializing behind pair 10's descriptor
                    nc.scalar.dma_start(
                        out=out[:, pair * 2 * SUB : pair * 2 * SUB + S, :], in_=zo
                    )
                    continue
                zo = opool.tile([128, 2 * SUB, C], BF16, tag="zo", name=f"zo{pair}")
                if g == len(GROUPS) - 2 and j >= g1 - g0 - 2:
                    sub_engine = nc.vector  # last full pair: DVE is freeing up
                else:
                    sub_engine = nc.gpsimd  # mid-stream: keep DVE for relu
                sub_engine.tensor_tensor(
                    out=zo,
                    in0=t_g[:, j : j + 2].rearrange("p a s c -> p (a s) c"),
                    in1=_bcast_cols(ls_g[:, j * SUB : (j + 2) * SUB], C),
                    op=mybir.AluOpType.subtract,
                )
                q0 = pair * 2 * SUB
                # spread the three final store descriptors (p10, p11, tail)
                # over sync/scalar/sync — all on scalar they serialized
                # 3 x 0.64us at the very end of the kernel
                store_engine = nc.scalar if pair == 11 else nc.sync
                store_engine.dma_start(out=out[:, q0 : q0 + 2 * SUB, :], in_=zo)

        for p in range(NPAIR):
            emit_pair_front(p)
            if p == NPAIR - 2:
                # hoist the half-block's MM1 ahead of MM2(p10): its relu
                # then completes well before MM2(tail) needs it
                emit_tail_front()
            if p >= 1:
                emit_back_pair(p - 1)
                g = _GROUP_OF[2 * (p - 1)]
                if 2 * (p - 1) + 1 == GROUPS[g][1] - 1:
                    emit_group_tail(g)
        emit_back_pair(NPAIR - 1)
        emit_group_tail(_GROUP_OF[2 * (NPAIR - 1)])
        emit_back_tail()
        emit_group_tail(_GROUP_OF[NBLK - 1])

    nc.compile()
    return nc


_NC = None


def _get_nc():
    global _NC
    if _NC is None:
        _NC = build_nc()
    return _NC


def make_in_maps(x, W1, b1, W2, b2):
    x = np.asarray(x, dtype=np.float32)
    # W1 [512, 256] -> [p, k, hid] (x16); W2 [256, 50] -> [p, kh, C] (x4)
    W1p = np.ascontiguousarray(
        (np.asarray(W1, dtype=np.float32) * W1_SCALE)
        .astype(np_fp8)
        .reshape(KC, 128, HID)
        .transpose(1, 0, 2)
    )
    W2p = np.ascontiguousarray(
        (np.asarray(W2, dtype=np.float32) * W2_SCALE)
        .astype(np_fp8)
        .reshape(MH, 128, C)
        .transpose(1, 0, 2)
    )
    # biases packed [p, MH + SUB*C]: 16*b1 columns then b2 tiled
    b1t = (np.asarray(b1, dtype=np.float32) * W1_SCALE).reshape(MH, 128).T
    b2t = np.tile(np.asarray(b2, dtype=np.float32), (128, SUB))
    bc = np.ascontiguousarray(np.concatenate([b1t, b2t], axis=1))

    in_maps = []
    full = NPAIR * 2 * BLOCK
    for i in range(N_CORES):
        r0 = i * ROWS_PER_CORE
        r1 = min(r0 + ROWS_PER_CORE, N_NODES)
        shard = np.zeros((ROWS_PER_CORE, F_IN), dtype=np_fp8)
        shard[: r1 - r0] = x[r0:r1].astype(np_fp8)
        # [rows, feat] -> [pair, p, sub, k, r] (contiguous per partition)
        xt = np.ascontiguousarray(
            shard[:full]
            .reshape(NPAIR, 2, BLOCK, KC, 128)
            .transpose(0, 4, 1, 3, 2)
        )
        # tail half block: [r, feat] -> [p, k, r]
        xtt = np.ascontiguousarray(
            shard[full:].reshape(TAIL_ROWS, KC, 128).transpose(2, 1, 0)
        )
        in_maps.append({"xT": xt, "xTt": xtt, "W1p": W1p, "W2p": W2p, "bc": bc})
    return in_maps


def run(x, W1, b1, W2, b2, trace=False, **spmd_kwargs):
    nc = _get_nc()
    in_maps = make_in_maps(x, W1, b1, W2, b2)
    res = run_bass_kernel_spmd(
        nc, in_maps, core_ids=list(range(N_CORES)), trace=trace, **spmd_kwargs
    )
    outs = []
    for i in range(N_CORES):
        o = np.asarray(res.results[i]["out"])  # [128, 98, 50] bf16, row = q*128+p
        outs.append(o.transpose(1, 0, 2).reshape(ROWS_PER_CORE, C))
    full = np.concatenate(outs, axis=0)[:N_NODES]
    return np.ascontiguousarray(full.astype(np.float32)), res


def kernel(x, edge_index, W1, b1, W2, b2):
    out, _ = run(x, W1, b1, W2, b2, trace=False)
    return out


# revision 55
# speedup vs baseline: 1.0674x; 1.0315x over previous
"""APPNP_Net Trainium2 kernel (8 NeuronCores, SPMD row-sharded), fp8 edition.

The reference model is:
    h = relu(x @ W1 + b1); z = h @ W2 + b2; out = log_softmax(z, axis=1)
followed by K=10 APPNP propagation steps with ALPHA=1.0.  Since
z_{t+1} = (1-ALPHA)*agg + ALPHA*h == h, the propagation is the identity
and edge_index never affects the output.  So the kernel is a row-wise
MLP + log_softmax, sharded by nodes across the 8 cores.

v4 changes (52.7us -> target ~46us), from NTFF trace analysis of v3:
  - x is packed block-major on the host ([pair, sub, p, k, r]), so every
    per-block DMA slice moves 1-2KB contiguous partition lines instead
    of the 512B strided runs that made block0's k01 land at 11.2us.
  - head descriptors split across both HWDGE queues: scalar writes
    block0-k01's descriptor (its first op, before the ACT_TABLE_LOAD),
    sync writes W1 / block0-k23 / biases / block1 / pair1.  ~2us earlier
    first matmul.
  - 10 warm-up matmuls (16 overshot: the last ones delayed the real
    stream; HAM's SHORT window kept resetting across the data-wait gap).
  - 12 x bufs (6 pairs in flight): v3's 1.9+0.9us PE gaps at 41-44us
    were the DMA rate-matching the shallow ring, then falling behind.
  - z-chain fused per PAIR: both blocks' MM2 outputs land in ONE psum
    bank ([128,2,4,50] = 1.6KB), so scale+bias / exp / row-sum /
    broadcast-subtract each run once per pair on 2x the elements —
    per-op overhead (~0.1-0.3us each on ACT/DVE/GpSimd) halves.
    ACT busy 31.4 -> ~26, DVE 31.5 -> ~24 predicted.

Numerics (unchanged): fp8e4 DoubleRow MM1 with W1'=16*W1, b1'=16*b1;
MM2 is plain fp8 (FD=50 < 128 makes DoubleRow's interleaved LDWEIGHTS a
net loss; FWL loads are ~50ns) with W2'=4*W2 so pz=64*(z-b2); t = pz/64
+ b2 fused in one DVE scalar_tensor_tensor; ACT table pinned; x fp8,
outputs bf16.  Rel err ~3.7e-3 vs the 2e-2 gate.
"""

import sys

sys.path.insert(0, "/opt/trn_rl_repo")

import dataclasses
from contextlib import ExitStack

import numpy as np
import ml_dtypes

import bass_rust as _bass_rust

import concourse.tile as tile
from concourse import bacc, mybir
from concourse.bass_utils import run_bass_kernel_spmd
from concourse.hw_specs import get_activation_tables

# NOTE on two dead ends, for future sessions: (1) walrus
# --enable-ldw-opt=true (redundant-LDWEIGHTS dedupe) rejects bass kernels
# (bass always emits standalone InstLdweights).  (2) capping walrus
# --max-sem-num to shrink the ~8.1us end-of-kernel semaphore-clear epilogue
# REGRESSES ~9us: fewer semaphores serialize the DMA queues, and the
# epilogue (a fixed 253-clear sweep split across the 5 engines) doesn't
# shrink.  The epilogue is ucode-fixed overhead inside the measured window.

N_NODES = 100000
F_IN = 512
HID = 256
C = 50
N_CORES = 8
BLOCK = 512
NBLK = 25  # 24 full blocks + 1 half block
NPAIR = 12  # full-block pairs
ROWS_PER_CORE = 12544  # 98 * 128; 8 * 12544 = 100352 >= 100000 (zero-padded)
TAIL_ROWS = ROWS_PER_CORE - NPAIR * 2 * BLOCK  # 256
# log-softmax groups (in blocks; boundaries must be even so pairs don't
# straddle groups): late groups are small so their serial ln/subtract
# chains overlap the remaining PE work instead of trailing it
GROUPS = [(0, 8), (8, 16), (16, 20), (20, 24), (24, 25)]
KC = F_IN // 128  # 4 contraction chunks for MM1
MH = HID // 128  # 2 hidden chunks
SUB = BLOCK // 128  # 4 row-subtiles per full block
NSUB = ROWS_PER_CORE // 128  # 98
N_WARM = 10  # dummy PE matmuls riding out the head DMA (~1.3us)

W1_SCALE = 16.0  # W1, b1 pre-scaled by this on host (fp8 normal range)
W2_SCALE = 4.0  # W2 pre-scaled by this on host
PZ_INV = 1.0 / (W1_SCALE * W2_SCALE)  # pz = 64*(z-b2); t = pz*PZ_INV + b2

BF16 = mybir.dt.bfloat16
F32 = mybir.dt.float32
FP8 = mybir.dt.float8e4
DR = mybir.MatmulPerfMode.DoubleRow
np_bf16 = ml_dtypes.bfloat16
np_fp8 = ml_dtypes.float8_e4m3

_GROUP_OF = {}
for _gi, (_s, _e) in enumerate(GROUPS):
    for _b in range(_s, _e):
        _GROUP_OF[_b] = _gi


def _bcast_cols(ap2d, reps):
    """[P, Q] AP -> [P, Q, reps] AP with a zero-stride inner dim."""
    return dataclasses.replace(ap2d, ap=[ap2d.ap[0], ap2d.ap[1], [0, reps]])


def _rep_dim(ap3d, reps):
    """[P, A, B] AP -> [P, reps, A, B] AP with a zero-stride leading dim."""
    return dataclasses.replace(
        ap3d, ap=[ap3d.ap[0], [0, reps], ap3d.ap[1], ap3d.ap[2]]
    )


def _pin_act_table(nc):
    """Constrain the ACT-table placement pass to natural_log_exp_and_others,
    which serves Relu, Exp AND Ln at full (400-bucket) resolution.  The
    default fixpoint picks exp_and_others for the steady Relu/Exp loop and
    switches tables around every Ln, costing ~1.3us per ACT_TABLE_LOAD; one
    table means one load.  Indices into the full list are preserved (the
    emitted act_func_set_id indexes act_info.json's act_func_sets)."""

    def patched():
        tables = [
            (name, funcs if name == "natural_log_exp_and_others" else set())
            for name, funcs in get_activation_tables(nc.m.arch).items()
        ]
        _bass_rust.insert_act_table_loads(nc, tables)

    nc.insert_act_table_loads = patched


def build_nc():
    nc = bacc.Bacc(
        "TRN2",
        target_bir_lowering=False,
        debug=False,
        num_devices=N_CORES,
    )
    _pin_act_table(nc)
    # [pair, p, sub, k, r]: a whole-pair transfer is 4KB contiguous per
    # partition line, per-block head slices are 1-2KB contiguous, and no
    # permuted DMA access patterns are needed anywhere
    xT = nc.declare_dram_parameter(
        "xT", [NPAIR, 128, 2, KC, BLOCK], FP8, isOutput=False
    ).ap()
    xTt = nc.declare_dram_parameter(
        "xTt", [128, KC, TAIL_ROWS], FP8, isOutput=False
    ).ap()
    # W1 packed [p, k, hid] (x16); W2 packed [p, kh, C] (x4); biases
    # packed [p, MH + SUB*C] (16*b1 columns then b2 broadcast)
    W1p = nc.declare_dram_parameter("W1p", [128, KC, HID], FP8, isOutput=False).ap()
    W2p = nc.declare_dram_parameter("W2p", [128, MH, C], FP8, isOutput=False).ap()
    bc = nc.declare_dram_parameter("bc", [128, MH + SUB * C], F32, isOutput=False).ap()
    # out[p, q, c] with row = q*128 + p (host transposes back)
    out = nc.declare_dram_parameter("out", [128, NSUB, C], BF16, isOutput=True).ap()

    with tile.TileContext(nc) as tc, ExitStack() as ctx:
        consts = ctx.enter_context(tc.tile_pool(name="consts", bufs=1))
        # 13 x bufs: pairs 0-11 + the tail each get a slot, so every x
        # descriptor can be written before the first out-store descriptor.
        # Store descriptors carry semaphore WAITS and block the sync queue
        # until their subtract fires — pair 11's x descriptor emitted after
        # them executed only at ~31us, starving the PE at ~36us for 1.7us.
        xpool = ctx.enter_context(tc.tile_pool(name="x", bufs=13))
        hpool = ctx.enter_context(tc.tile_pool(name="h", bufs=5))
        tpool = ctx.enter_context(tc.tile_pool(name="t", bufs=3))
        epool = ctx.enter_context(tc.tile_pool(name="e", bufs=3))
        spool = ctx.enter_context(tc.tile_pool(name="s", bufs=2))
        opool = ctx.enter_context(tc.tile_pool(name="o", bufs=4))
        # ph0/ph1 rings hold MM1 accumulators (6 banks); the pz ring holds
        # MM2 pair-outputs (1 bank each: 2*4*50 fp32 = 1.6KB) + warm-up
        psum = ctx.enter_context(tc.tile_pool(name="psum", bufs=3, space="PSUM"))
        psumz = ctx.enter_context(tc.tile_pool(name="psumz", bufs=2, space="PSUM"))

        # ── PE warm-up ────────────────────────────────────────────────
        # HAM throttles an idle PE to 1.2 GHz and needs ~3.4us of sustained
        # matmul activity to lift.  Burn the head's DMA wait with dummy DR
        # matmuls on a memset scratch so the real MM1 stream starts warm-ish.
        warm = consts.tile([128, 2, 128], FP8, tag="warm")
        nc.vector.memset(warm, 0.25)
        wps = psumz.tile([128, 128], F32, tag="pz", name="warm_ps")
        for _ in range(N_WARM):
            nc.tensor.matmul(
                wps, lhsT=warm, rhs=warm, start=True, stop=True, perf_mode=DR
            )

        # ── head DMAs, split across both HWDGE queues ─────────────────
        # DIRECT2D descriptor writes cost ~0.65us each and serialize per
        # engine.  scalar's first op is block0-k01's descriptor (the first
        # matmul's gating data, in flight by ~8us); sync carries W1 then
        # the rest in need-order.  gpsimd's SWDGE queue stays unused.
        # sync carries the x stream in need-order (k01, k23, block1,
        # pair1, ...) so every chunk's descriptor lands ~0.65us earlier
        # than with W1 in front; scalar carries the small consts (W1
        # 128KB, biases, W2) whose transfers finish almost instantly.
        xts = {}
        xt0 = xpool.tile([128, 2, KC, BLOCK], FP8, tag="xt", name="xt0")
        nc.sync.dma_start(out=xt0[:, 0, :2, :], in_=xT[0, :, 0, :2, :])
        w1t = consts.tile([128, KC, HID], FP8, tag="w1")
        nc.scalar.dma_start(out=w1t, in_=W1p)
        nc.sync.dma_start(out=xt0[:, 0, 2:, :], in_=xT[0, :, 0, 2:, :])
        bct = consts.tile([128, MH + SUB * C], F32, tag="bc")
        nc.scalar.dma_start(out=bct, in_=bc)
        nc.sync.dma_start(out=xt0[:, 1], in_=xT[0, :, 1])
        b1sb = bct[:, :MH]
        b2sb = bct[:, MH:].rearrange("p (s c) -> p s c", s=SUB)
        w2t = consts.tile([128, MH, C], FP8, tag="w2")
        nc.scalar.dma_start(out=w2t, in_=W2p)
        xts[0] = xt0
        xt1 = xpool.tile([128, 2, KC, BLOCK], FP8, tag="xt", name="xt1")
        # per-block: MM1(pair1) starts on block 2's data alone instead of
        # waiting the whole 512KB pair (a consistent ~0.9us PE gap at ~12us
        # that also reset HAM's warm-up window).  Only xt1 is split — the
        # xt1+xt2 variant's extra descriptors pushed later ones back.
        nc.sync.dma_start(out=xt1[:, 0], in_=xT[1, :, 0])
        nc.sync.dma_start(out=xt1[:, 1], in_=xT[1, :, 1])
        xts[1] = xt1

        def issue_pair(p):
            if p == NPAIR:
                xt = xpool.tile([128, KC, TAIL_ROWS], FP8, tag="xt", name="xt_tail")
                nc.sync.dma_start(out=xt, in_=xTt)
            else:
                xt = xpool.tile([128, 2, KC, BLOCK], FP8, tag="xt", name=f"xt{p}")
                nc.sync.dma_start(out=xt, in_=xT[p])
            xts[p] = xt

        hs = {}  # block -> h tile [128, MH, BLOCK] fp8 (16*relu(...))
        t_gs = {}  # group -> t tile
        s_gs = {}  # group -> s tile

        # With the ACT table pinned (no Ln switches), relu routing is pure
        # load balance: ACT takes mh0 always, plus mh1 on a few blocks so
        # ACT (~relu0+exp+ln) and DVE (~relu1+scale-bias+reduce) even out.
        # block 23's relu on ACT relieves DVE right where MM2(p11)/MM2(tail)
        # were stalling ~1.3us on relu semaphores at the end of the stream
        act_mh1 = {5, 9, 13, 23}

        def emit_relu(b, mh, ph, R):
            ht = hs[b]
            if mh == 0 or b in act_mh1:
                # ScalarE: 16h = relu(ph + 16*b1)
                nc.scalar.activation(
                    ht[:, mh, :R],
                    ph[:, :R],
                    mybir.ActivationFunctionType.Relu,
                    bias=b1sb[:, mh : mh + 1],
                )
            else:
                # VectorE: (ph + 16*b1) max 0 — balance the engines
                nc.vector.tensor_scalar(
                    out=ht[:, mh, :R],
                    in0=ph[:, :R],
                    scalar1=b1sb[:, mh : mh + 1],
                    scalar2=0.0,
                    op0=mybir.AluOpType.add,
                    op1=mybir.AluOpType.max,
                )

        def emit_pair_front(p):
            """x prefetch, MM1 (fp8 DR, mh->k2->block), relu for pair p."""
            # throttled prefetch: 2 pairs per iteration early on — issuing
            # 6 descriptors at p=0 made pairs 3-7's transfers round-robin-
            # steal queue bandwidth from the urgently needed xt1/xt2 (the
            # 1.1-1.8us PE gaps at 10-16us).  By p=4 everything (incl. pair
            # 11 + tail) is issued, still ahead of the first store
            # descriptor whose semaphore wait would block the sync queue.
            hi = NPAIR + 1 if p >= 4 else 2 * p + 4
            for pf in range(2, hi):
                if pf not in xts:
                    issue_pair(pf)
            xt = xts[p]
            blocks = [2 * p, 2 * p + 1]
            for b in blocks:
                hs[b] = hpool.tile([128, MH, BLOCK], FP8, tag="h", name=f"h{b}")
            for mh in range(MH):
                phs = [
                    psum.tile([128, BLOCK], F32, tag=f"ph{mh}", name=f"ph{mh}_{b}")
                    for b in blocks
                ]
                for k2 in range(KC // 2):
                    for s in range(2):
                        nc.tensor.matmul(
                            phs[s],
                            lhsT=w1t[:, 2 * k2 : 2 * k2 + 2, mh * 128 : (mh + 1) * 128],
                            rhs=xt[:, s, 2 * k2 : 2 * k2 + 2, :],
                            start=(k2 == 0),
                            stop=(k2 == KC // 2 - 1),
                            perf_mode=DR,
                        )
                for s, b in enumerate(blocks):
                    emit_relu(b, mh, phs[s], BLOCK)
                if p < 2:
                    # bridge the early x-DMA gaps (pair1/2 land ~12-14us)
                    # with dependency-free dummies so HAM's 3.4us
                    # continuous-activity window isn't reset — each reset
                    # costs ~1.5-3us of half-clock PE time; a dummy costs
                    # ~107ns only if the real data was late anyway
                    for _ in range(3):
                        nc.tensor.matmul(
                            wps, lhsT=warm, rhs=warm,
                            start=True, stop=True, perf_mode=DR,
                        )

        def emit_tail_front():
            """MM1 + relu for the final half block (256 rows)."""
            b = NBLK - 1
            R = TAIL_ROWS
            if NPAIR not in xts:
                issue_pair(NPAIR)
            xt = xts[NPAIR]
            hs[b] = hpool.tile([128, MH, BLOCK], FP8, tag="h", name=f"h{b}")
            for mh in range(MH):
                ph = psum.tile([128, BLOCK], F32, tag=f"ph{mh}", name=f"ph{mh}_t")
                for k2 in range(KC // 2):
                    nc.tensor.matmul(
                        ph[:, :R],
                        lhsT=w1t[:, 2 * k2 : 2 * k2 + 2, mh * 128 : (mh + 1) * 128],
                        rhs=xt[:, 2 * k2 : 2 * k2 + 2, :R],
                        start=(k2 == 0),
                        stop=(k2 == KC // 2 - 1),
                        perf_mode=DR,
                    )
                emit_relu(b, mh, ph, R)

        def _group_tiles(g):
            g0, g1 = GROUPS[g]
            glen = g1 - g0
            t_gs[g] = tpool.tile([128, glen, SUB, C], BF16, tag="t", name=f"t_g{g}")
            s_gs[g] = spool.tile([128, glen * SUB], BF16, tag="s", name=f"s_g{g}")

        def emit_back_pair(p):
            """MM2 (both blocks into ONE psum bank), then pair-granular
            scale+bias, exp, row-sum.  Halves the per-op overhead vs
            per-block ops."""
            b0 = 2 * p
            g = _GROUP_OF[b0]
            g0, g1 = GROUPS[g]
            j = b0 - g0
            if j == 0:
                _group_tiles(g)
            t_g, s_g = t_gs[g], s_gs[g]
            pz = psumz.tile([128, 2, SUB, C], F32, tag="pz", name=f"pz{p}")
            for s in range(2):
                ht = hs.pop(b0 + s)
                # MM2 runs non-DR: at FD=50 DoubleRow's interleaved
                # LDWEIGHTS (~140ns) loses to FWL (~50ns); two accumulating
                # 128-contraction matmuls per subtile
                for rs in range(SUB):
                    for mh in range(MH):
                        nc.tensor.matmul(
                            pz[:, s, rs, :],
                            lhsT=ht[:, mh, rs * 128 : (rs + 1) * 128],
                            rhs=w2t[:, mh, :],
                            start=(mh == 0),
                            stop=(mh == MH - 1),
                        )
            # t = z = pz/64 + b2 for both blocks in one DVE op
            nc.vector.scalar_tensor_tensor(
                out=t_g[:, j : j + 2],
                in0=pz,
                scalar=PZ_INV,
                in1=_rep_dim(b2sb, 2),
                op0=mybir.AluOpType.mult,
                op1=mybir.AluOpType.add,
            )
            e = epool.tile([128, 2, SUB, C], BF16, tag="e", name=f"e{p}")
            nc.scalar.activation(
                e, t_g[:, j : j + 2], mybir.ActivationFunctionType.Exp
            )
            with nc.allow_low_precision(reason="bf16 softmax row-sum"):
                nc.vector.reduce_sum(
                    out=s_g[:, j * SUB : (j + 2) * SUB].rearrange(
                        "p (a s) -> p a s", a=2
                    ),
                    in_=e,
                    axis=mybir.AxisListType.X,
                )

        def emit_back_tail():
            """MM2 + z-chain for the final half block."""
            b = NBLK - 1
            S = TAIL_ROWS // 128
            g = _GROUP_OF[b]
            g0, _ = GROUPS[g]
            j = b - g0
            if j == 0:
                _group_tiles(g)
            t_g, s_g = t_gs[g], s_gs[g]
            ht = hs.pop(b)
            pz = psumz.tile([128, SUB, C], F32, tag="pz", name="pz_t")
            for rs in range(S):
                for mh in range(MH):
                    nc.tensor.matmul(
                        pz[:, rs, :],
                        lhsT=ht[:, mh, rs * 128 : (rs + 1) * 128],
                        rhs=w2t[:, mh, :],
                        start=(mh == 0),
                        stop=(mh == MH - 1),
                    )
            nc.vector.scalar_tensor_tensor(
                out=t_g[:, j, :S, :],
                in0=pz[:, :S, :],
                scalar=PZ_INV,
                in1=b2sb[:, :S, :],
                op0=mybir.AluOpType.mult,
                op1=mybir.AluOpType.add,
            )
            e = epool.tile([128, SUB, C], BF16, tag="e", name="e_t")
            nc.scalar.activation(
                e[:, :S, :], t_g[:, j, :S, :], mybir.ActivationFunctionType.Exp
            )
            with nc.allow_low_precision(reason="bf16 softmax row-sum"):
                nc.vector.reduce_sum(
                    out=s_g[:, j * SUB : j * SUB + S],
                    in_=e[:, :S, :],
                    axis=mybir.AxisListType.X,
                )

        def emit_group_tail(g):
            """ls = ln(s); out = z - ls (class-broadcast) per pair; store."""
            g0, g1 = GROUPS[g]
            t_g, s_g = t_gs.pop(g), s_gs.pop(g)
            last = g1 == NBLK
            ncols = (g1 - g0 - 1) * SUB + (TAIL_ROWS // 128 if last else SUB)
            ls_g = spool.tile(
                [128, (g1 - g0) * SUB], BF16, tag="ls", name=f"ls_g{g}"
            )
            nc.scalar.activation(
                ls_g[:, :ncols], s_g[:, :ncols], mybir.ActivationFunctionType.Ln
            )
            for j in range(0, g1 - g0, 2):
                b = g0 + j
                pair = b // 2
                if b == NBLK - 1:  # half-block tail, alone in its group
                    S = TAIL_ROWS // 128
                    zo = opool.tile([128, S, C], BF16, tag="zo", name="zo_t")
                    nc.vector.tensor_tensor(
                        out=zo,
                        in0=t_g[:, j, :S, :],
                        in1=_bcast_cols(ls_g[:, j * SUB : j * SUB + S], C),
                        op=mybir.AluOpType.subtract,
                    )
                    # sync is idle by now; scalar carries pair 11's store
                    # scalar is idle after its last ln; the tail store on
                    # sync was serializing behind pair 10's descriptor
                    nc.scalar.dma_start(
                        out=out[:, pair * 2 * SUB : pair * 2 * SUB + S, :], in_=zo
                    )
                    continue
                zo = opool.tile([128, 2 * SUB, C], BF16, tag="zo", name=f"zo{pair}")
                if g == len(GROUPS) - 2 and j >= g1 - g0 - 2:
                    sub_engine = nc.vector  # last full pair: DVE is freeing up
                else:
                    sub_engine = nc.gpsimd  # mid-stream: keep DVE for relu
                sub_engine.tensor_tensor(
                    out=zo,
                    in0=t_g[:, j : j + 2].rearrange("p a s c -> p (a s) c"),
                    in1=_bcast_cols(ls_g[:, j * SUB : (j + 2) * SUB], C),
                    op=mybir.AluOpType.subtract,
                )
                q0 = pair * 2 * SUB
                # spread the three final store descriptors (p10, p11, tail)
                # over sync/scalar/sync — all on scalar they serialized
                # 3 x 0.64us at the very end of the kernel
                store_engine = nc.scalar if pair == 11 else nc.sync
                store_engine.dma_start(out=out[:, q0 : q0 + 2 * SUB, :], in_=zo)

        for p in range(NPAIR):
            emit_pair_front(p)
            if p == NPAIR - 2:
                # hoist the half-block's MM1 ahead of MM2(p10): its relu
                # then completes well before MM2(tail) needs it
                emit_tail_front()
            if p >= 1:
                emit_back_pair(p - 1)
                g = _GROUP_OF[2 * (p - 1)]
                if 2 * (p - 1) + 1 == GROUPS[g][1] - 1:
                    emit_group_tail(g)
        emit_back_pair(NPAIR - 1)
        emit_group_tail(_GROUP_OF[2 * (NPAIR - 1)])
        emit_back_tail()
        emit_group_tail(_GROUP_OF[NBLK - 1])

    nc.compile()
    return nc


_NC = None


def _get_nc():
    global _NC
    if _NC is None:
        _NC = build_nc()
    return _NC


def make_in_maps(x, W1, b1, W2, b2):
    x = np.asarray(x, dtype=np.float32)
    # W1 [512, 256] -> [p, k, hid] (x16); W2 [256, 50] -> [p, kh, C] (x4)
    W1p = np.ascontiguousarray(
        (np.asarray(W1, dtype=np.float32) * W1_SCALE)
        .astype(np_fp8)
        .reshape(KC, 128, HID)
        .transpose(1, 0, 2)
    )
    W2p = np.ascontiguousarray(
        (np.asarray(W2, dtype=np.float32) * W2_SCALE)
        .astype(np_fp8)
        .reshape(MH, 128, C)
        .transpose(1, 0, 2)
    )
    # biases packed [p, MH + SUB*C]: 16*b1 columns then b2 tiled
    b1t = (np.asarray(b1, dtype=np.float32) * W1_SCALE).reshape(MH, 128).T
    b2t = np.tile(np.asarray(b2, dtype=np.float32), (128, SUB))
    bc = np.ascontiguousarray(np.concatenate([b1t, b2t], axis=1))

    in_maps = []
    full = NPAIR * 2 * BLOCK
    for i in range(N_CORES):
        r0 = i * ROWS_PER_CORE
        r1 = min(r0 + ROWS_PER_CORE, N_NODES)
        shard = np.zeros((ROWS_PER_CORE, F_IN), dtype=np_fp8)
        shard[: r1 - r0] = x[r0:r1].astype(np_fp8)
        # [rows, feat] -> [pair, p, sub, k, r] (contiguous per partition)
        xt = np.ascontiguousarray(
            shard[:full]
            .reshape(NPAIR, 2, BLOCK, KC, 128)
            .transpose(0, 4, 1, 3, 2)
        )
        # tail half block: [r, feat] -> [p, k, r]
        xtt = np.ascontiguousarray(
            shard[full:].reshape(TAIL_ROWS, KC, 128).transpose(2, 1, 0)
        )
        in_maps.append({"xT": xt, "xTt": xtt, "W1p": W1p, "W2p": W2p, "bc": bc})
    return in_maps


def run(x, W1, b1, W2, b2, trace=False, **spmd_kwargs):
    nc = _get_nc()
    in_maps = make_in_maps(x, W1, b1, W2, b2)
    res = run_bass_kernel_spmd(
        nc, in_maps, core_ids=list(range(N_CORES)), trace=trace, **spmd_kwargs
    )
    outs = []
    for i in range(N_CORES):
        o = np.asarray(res.results[i]["out"])  # [128, 98, 50] bf16, row = q*128+p
        outs.append(o.transpose(1, 0, 2).reshape(ROWS_PER_CORE, C))
    full = np.concatenate(outs, axis=0)[:N_NODES]
    return np.ascontiguousarray(full.astype(np.float32)), res


def kernel(x, edge_index, W1, b1, W2, b2):
    out, _ = run(x, W1, b1, W2, b2, trace=False)
    return out
